# revision 1
# baseline (speedup 1.0000x reference)
"""GATv2 4-layer + MLP head on 8 Trainium2 NeuronCores (Bass/Tile), v2.

Strategy: partition destination nodes across 8 cores (1250 each, degree-
balanced into 10 blocks of 125). Per layer, per dst block:
  node : xl/xr for the block's 125 nodes via PE matmuls from the SBUF-resident
         feature-major h of the previous layer (produced by PE transposes);
         xl rows DMA to DRAM; every 2 blocks an AllGather chunk fires so the
         collective overlaps the remaining edge compute.
  edge : SWDGE-gather xl[src] rows (edge-major, bf16); xr[dst] rows broadcast
         to edges on the PE with a one-hot E01^T matmul (no second gather);
         u = xl[s]+xr[d]; e = att . leaky_relu(u) via two batched Prelu calls
         (alpha=0.2 on positive-att columns; alpha=5, scale=-1 on negative
         ones, 0.2|att| folded into the tables) + a DVE pool_avg row-reduce;
         p = exp(512*avg); aggregation via scatter matmul S = E01 * p into
         PSUM; h = relu(num/den).
Weight/bias tables are scaled on device (|att| folded in, signs via a host
column permutation; next layer unscales by 1/beta). MLP head feature-major;
softmax via exp-based sigmoid (stays in the exp ACT table set).
"""
import sys

sys.path.insert(0, "/opt/trn_rl_repo")

from contextlib import ExitStack

import numpy as np
import ml_dtypes

import concourse.bass as bass
import concourse.bacc as bacc
import concourse.tile as tile
from concourse import mybir
from concourse.bass_utils import run_bass_kernel_spmd

bf16 = mybir.dt.bfloat16
f32 = mybir.dt.float32
i16 = mybir.dt.int16
AF = mybir.ActivationFunctionType
ALU = mybir.AluOpType
ts = bass.ts
npbf = ml_dtypes.bfloat16

N, E, DIN, H = 10000, 80000, 1024, 512
NEG = 0.2
NC = 8
NLOC = N // NC          # 1250 dst nodes per core
BLK = 125               # dst nodes per block
NBLK = NLOC // BLK      # 10 blocks per core
CH = 5                  # AllGather chunks per layer
CROWS = NLOC // CH      # rows per AG chunk (250)
CBLK = NBLK // CH       # blocks per AG chunk (2)

# wblob row offsets (f32 [WROWS, 512]); weight cols padded to 512
OFF = {}
_o = 0
for _l in range(4):
    din = DIN if _l == 0 else H
    OFF[f"wl{_l}"] = _o; _o += din
    OFF[f"wr{_l}"] = _o; _o += din
OFF["lw1"] = _o; _o += H
OFF["lw2"] = _o; _o += H
OFF["lw3"] = _o; _o += 256
OFF["att"] = _o; _o += 4     # rows att l=0..3
OFF["bl"] = _o; _o += 4
OFF["br"] = _o; _o += 4
OFF["bb"] = _o; _o += 4
OFF["lb1"] = _o; _o += 1
OFF["lb2"] = _o; _o += 1
OFF["lb3"] = _o; _o += 1
OFF["negmask"] = _o; _o += 4  # 1.0 where att column is negative (post-perm)
WROWS = _o


# ---------------------------------------------------------------- host prep
def _prep_edges(edge_index):
    src = np.concatenate([edge_index[0], np.arange(N)]).astype(np.int64)
    dst = np.concatenate([edge_index[1], np.arange(N)]).astype(np.int64)
    deg = np.bincount(dst, minlength=N)
    NBUCK = NC * NBLK
    order = np.argsort(-deg, kind="stable")
    bucket_edges = np.zeros(NBUCK, np.int64)
    bucket_nodes = [[] for _ in range(NBUCK)]
    import heapq
    heap = [(0, kk) for kk in range(NBUCK)]
    heapq.heapify(heap)
    for g in order:
        while True:
            w, kk = heapq.heappop(heap)
            if len(bucket_nodes[kk]) < BLK:
                break
        bucket_nodes[kk].append(int(g))
        bucket_edges[kk] = w + int(deg[g])
        if len(bucket_nodes[kk]) < BLK:
            heapq.heappush(heap, (int(bucket_edges[kk]), kk))
    assign = [[] for _ in range(NC)]
    for c in range(NC):
        for b in range(NBLK):
            assign[c].extend(bucket_nodes[c * NBLK + b])
    assign = [np.array(a, np.int64) for a in assign]
    # chunked AG row layout: node at (core c, pos p) lands at
    # (p//CROWS)*(NC*CROWS) + c*CROWS + p%CROWS in xl_full [N, H]
    rowof = np.empty(N, np.int64)
    posof = np.empty(N, np.int64)
    coreof = np.empty(N, np.int64)
    for c in range(NC):
        p = np.arange(NLOC)
        rowof[assign[c]] = (p // CROWS) * (NC * CROWS) + c * CROWS + (p % CROWS)
        posof[assign[c]] = p
        coreof[assign[c]] = c
    percore = []
    for c in range(NC):
        sel = coreof[dst] == c
        s_, d_ = rowof[src[sel]], posof[dst[sel]]
        o = np.argsort(d_, kind="stable")
        s_, d_ = s_[o], d_[o]
        blocks = []
        for bb in range(NBLK):
            m = (d_ // BLK) == bb
            blocks.append((s_[m], d_[m] - bb * BLK))
        percore.append(blocks)
    TBs = tuple(max(max(-(-len(percore[c][b][0]) // 128), 1) for c in range(NC))
                for b in range(NBLK))
    cum = np.concatenate([[0], np.cumsum(TBs)]).astype(int)
    NT = int(cum[-1])
    EPAD = NT * 128
    cores = []
    for c in range(NC):
        src16 = np.zeros(EPAD, np.int16)
        e01 = np.zeros((128, NT * BLK), npbf)
        e01T = np.zeros((128, NT * 128), npbf)
        for b in range(NBLK):
            s, d = percore[c][b]
            n = len(s)
            base = int(cum[b]) * 128
            src16[base:base + n] = s
            tt = int(cum[b]) + np.arange(n) // 128
            pp = np.arange(n) % 128
            e01[pp, tt * BLK + d] = 1.0
            e01T[d, tt * 128 + pp] = 1.0
        w = src16.reshape(-1, 16).T.copy()
        cores.append(dict(src16=np.tile(w, (8, 1)).copy(),
                          e01=np.ascontiguousarray(e01),
                          e01T=np.ascontiguousarray(e01T)))
    return TBs, cores, assign


def _pack_wblob(inp, perms):
    """Pack all fp32 weights/biases into one [WROWS, 512] f32 array.
    Only index permutations happen here (no float arithmetic)."""
    wb = np.zeros((WROWS, H), np.float32)
    for l in range(4):
        rowp = perms[l - 1] if l > 0 else None
        for nm in ("wl", "wr"):
            W = np.asarray(inp[f"{nm}{l + 1}"], np.float32)
            if rowp is not None:
                W = W[rowp, :]
            W = W[:, perms[l]]
            nk = W.shape[0] // 128
            W = W.reshape(nk, 128, H).transpose(1, 0, 2).reshape(nk * 128, H)
            wb[OFF[f"{nm}{l}"]:OFF[f"{nm}{l}"] + W.shape[0], :] = W
    lw1 = np.asarray(inp["lw1"], np.float32)[perms[3], :]
    wb[OFF["lw1"]:OFF["lw1"] + H, :] = lw1
    wb[OFF["lw2"]:OFF["lw2"] + H, :256] = np.asarray(inp["lw2"], np.float32)
    wb[OFF["lw3"]:OFF["lw3"] + 256, :2] = np.asarray(inp["lw3"], np.float32)
    for l in range(4):
        wb[OFF["att"] + l, :] = np.asarray(inp[f"att{l + 1}"], np.float32)[perms[l]]
        wb[OFF["bl"] + l, :] = np.asarray(inp[f"bl{l + 1}"], np.float32)[perms[l]]
        wb[OFF["br"] + l, :] = np.asarray(inp[f"br{l + 1}"], np.float32)[perms[l]]
        wb[OFF["bb"] + l, :] = np.asarray(inp[f"b{l + 1}"], np.float32)[perms[l]]
    wb[OFF["lb1"], :] = np.asarray(inp["lb1"], np.float32)
    wb[OFF["lb2"], :256] = np.asarray(inp["lb2"], np.float32)
    wb[OFF["lb3"], :2] = np.asarray(inp["lb3"], np.float32)
    for l in range(4):
        kp = int((np.asarray(inp[f"att{l + 1}"], np.float32) > 0).sum())
        wb[OFF["negmask"] + l, kp:] = 1.0
    return wb


# -------------------------------------------------------------- bass program
def _build(TBs, KP, single_core=False, nlayers=4, nedge=True, reps=1):
    TBs = tuple(TBs)
    TBMAX = max(TBs)
    cum = [0]
    for t in TBs:
        cum.append(cum[-1] + t)
    NT = cum[-1]
    nc = bacc.Bacc("TRN2", num_swdge_queues=2)
    P = nc.declare_dram_parameter
    x_in = P("x", [DIN, NLOC], f32, isOutput=False)
    wb_in = P("wblob", [WROWS, H], f32, isOutput=False)
    srcidx_in = P("srcidx", [128, NT * 8], i16, isOutput=False)
    e01_in = P("e01", [128, NT * BLK], bf16, isOutput=False)
    e01T_in = P("e01T", [128, NT * 128], bf16, isOutput=False)
    ident_in = P("ident", [128, 128], bf16, isOutput=False)
    logitsT_out = P("logitsT", [2, NLOC], f32, isOutput=True)
    probs0_out = P("probs0", [1, NLOC], f32, isOutput=True)
    probs1_out = P("probs1", [1, NLOC], f32, isOutput=True)

    beta_dr = nc.dram_tensor("beta_dr", [4, H], f32)
    xl_loc = [nc.dram_tensor(f"xlloc{l}", [NLOC, H], bf16) for l in range(4)]
    xl_full = [nc.dram_tensor(f"xlfull{l}", [N, H], bf16, addr_space="Shared")
               for l in range(4)]

    with tile.TileContext(nc) as tc, ExitStack() as ctx:
        wp = ctx.enter_context(tc.tile_pool(name="wp", bufs=1))
        np_ = ctx.enter_context(tc.tile_pool(name="np", bufs=2))
        np2 = ctx.enter_context(tc.tile_pool(name="np2", bufs=2))
        ep = ctx.enter_context(tc.tile_pool(name="ep", bufs=2))
        gp = ctx.enter_context(tc.tile_pool(name="gp", bufs=2))
        ps = ctx.enter_context(tc.tile_pool(name="ps", bufs=1, space="PSUM"))
        ps2 = ctx.enter_context(tc.tile_pool(name="ps2", bufs=2, space="PSUM"))

        # ---------------- constants ----------------
        ones128 = wp.tile([128, 1], bf16, tag="ones128")
        nc.vector.memset(ones128[:, :], 1.0)
        onesrow = wp.tile([1, 128], bf16, tag="onesrow")
        nc.vector.memset(onesrow[:1, :], 1.0)
        sgn = wp.tile([2, 1], f32, tag="sgn")
        nc.vector.memset(sgn[:2, :], 1.0)
        nc.vector.tensor_scalar_mul(sgn[0:1, :], sgn[0:1, :], -1.0)
        ident = wp.tile([128, 128], bf16, tag="ident")
        nc.scalar.dma_start(out=ident[:, :], in_=ident_in[:, :])
        e01_sb = wp.tile([128, NT * BLK], bf16, tag="e01")
        nc.scalar.dma_start(out=e01_sb[:, :], in_=e01_in[:, :])
        e01T_sb = wp.tile([128, NT * 128], bf16, tag="e01T")
        nc.scalar.dma_start(out=e01T_sb[:, :], in_=e01T_in[:, :])
        srcidx = wp.tile([128, NT * 8], i16, tag="srcidx")
        nc.scalar.dma_start(out=srcidx[:, :], in_=srcidx_in[:, :])

        # ---------------- beta / biases ----------------
        beta = wp.tile([4, H], f32, tag="beta")
        nc.scalar.dma_start(out=beta[:4, :], in_=wb_in[OFF["att"]:OFF["att"] + 4, :])
        nc.scalar.activation(beta[:4, :], beta[:4, :], AF.Abs)
        nmask = np_.tile([4, H], f32, tag="brow", bufs=2)
        nc.scalar.dma_start(out=nmask[:4, :],
                          in_=wb_in[OFF["negmask"]:OFF["negmask"] + 4, :])
        nc.vector.tensor_scalar_mul(nmask[:4, :], nmask[:4, :], NEG - 1.0)
        nc.vector.tensor_scalar_add(nmask[:4, :], nmask[:4, :], 1.0)
        nc.vector.tensor_mul(beta[:4, :], beta[:4, :], nmask[:4, :])
        nc.vector.tensor_scalar_max(beta[:4, :], beta[:4, :], 1e-30)
        nc.sync.dma_start(out=beta_dr[:, :], in_=beta[:4, :])
        brow = np_.tile([4, H], f32, tag="brow", bufs=2)
        nc.scalar.dma_start(out=brow[:4, :], in_=wb_in[OFF["bl"]:OFF["bl"] + 4, :])
        brow2 = np_.tile([4, H], f32, tag="brow2", bufs=1)
        nc.scalar.dma_start(out=brow2[:4, :], in_=wb_in[OFF["bb"]:OFF["bb"] + 4, :])
        nc.vector.tensor_add(brow[:4, :], brow[:4, :], brow2[:4, :])
        blb4 = np_.tile([4, H], bf16, tag="blb4", bufs=1)
        nc.vector.tensor_mul(blb4[:4, :], brow[:4, :], beta[:4, :])
        brow3 = np_.tile([4, H], f32, tag="brow", bufs=2)
        nc.scalar.dma_start(out=brow3[:4, :], in_=wb_in[OFF["br"]:OFF["br"] + 4, :])
        nc.vector.tensor_sub(brow3[:4, :], brow3[:4, :], brow2[:4, :])
        brb4 = np_.tile([4, H], bf16, tag="brb4", bufs=1)
        nc.vector.tensor_mul(brb4[:4, :], brow3[:4, :], beta[:4, :])
        # matmul operands must sit at partition base 0: one [1, H] tile per row
        bias_dr = nc.dram_tensor("bias_dr", [8, H], bf16)
        nc.sync.dma_start(out=bias_dr[0:4, :], in_=blb4[:4, :])
        nc.sync.dma_start(out=bias_dr[4:8, :], in_=brb4[:4, :])
        blb_row, brb_row = [], []
        for l in range(4):
            t = wp.tile([1, H], bf16, tag=f"blb{l}")
            nc.sync.dma_start(out=t[:1, :], in_=bias_dr[l:l + 1, :])
            blb_row.append(t)
            t = wp.tile([1, H], bf16, tag=f"brb{l}")
            nc.sync.dma_start(out=t[:1, :], in_=bias_dr[4 + l:5 + l, :])
            brb_row.append(t)

        # recipcol[l]: [128, 4] = 1/beta_l in (k p) layout (used by layer l+1, lw1)
        recipcol = []
        for l in range(4):
            rc = wp.tile([128, H // 128], f32, tag=f"rc{l}")
            nc.sync.dma_start(out=rc[:, :],
                              in_=beta_dr[l, :].rearrange("(k p) -> p k", p=128))
            rcr = wp.tile([128, H // 128], f32, tag=f"rcr{l}")
            nc.vector.reciprocal(rcr[:, :], rc[:, :])
            recipcol.append(rcr)

        # ---------------- GAT weight prep (into SBUF, rotation bufs=2) -------
        wld_t, wrd_t = [], []
        for l in range(4):
            din = DIN if l == 0 else H
            nk0 = din // 128
            attb = np_.tile([128, H], f32, tag="attb", bufs=1)
            nc.sync.dma_start(out=attb[:, :],
                              in_=beta_dr[l:l + 1, :].broadcast_to((128, H)))
            wld = wp.tile([128, nk0, H], bf16, tag=f"wld{l}")
            wrd = wp.tile([128, nk0, H], bf16, tag=f"wrd{l}")
            wld_t.append(wld)
            wrd_t.append(wrd)
            for W_off, wdev in ((OFF[f"wl{l}"], wld), (OFF[f"wr{l}"], wrd)):
                slab = wb_in[W_off:W_off + nk0 * 128, :].rearrange(
                    "(p k) h -> p k h", p=128)
                for k0 in range(0, nk0, 4):
                    kw = min(4, nk0 - k0)
                    wt = np_.tile([128, 4, H], f32, tag="wstage", bufs=1)
                    nc.scalar.dma_start(out=wt[:, :kw, :], in_=slab[:, k0:k0 + kw, :])
                    for kk in range(kw):
                        k = k0 + kk
                        # 1/beta of the previous layer is applied in the hT
                        # transpose-copies, so only the column scale remains.
                        nc.vector.tensor_mul(wdev[:, k, :], wt[:, kk, :], attb[:, :])

        # ---------------- MLP weight prep ----------------
        lw1_dev = wp.tile([128, 4, H], bf16, tag="lw1")
        for k in range(4):
            wt = np_.tile([128, 4, H], f32, tag="wstage", bufs=1)
            nc.sync.dma_start(out=wt[:, 0, :], in_=wb_in[OFF["lw1"] + k * 128:
                                                         OFF["lw1"] + (k + 1) * 128, :])
            nc.gpsimd.tensor_copy(lw1_dev[:, k, :], wt[:, 0, :])
        lw2_dev = wp.tile([128, 4, 256], bf16, tag="lw2")
        for k in range(4):
            wt = np_.tile([128, 4, H], f32, tag="wstage", bufs=1)
            nc.sync.dma_start(out=wt[:, 0, :256], in_=wb_in[OFF["lw2"] + k * 128:
                                                            OFF["lw2"] + (k + 1) * 128, :256])
            nc.gpsimd.tensor_copy(lw2_dev[:, k, :], wt[:, 0, :256])
        lw3_dev = wp.tile([128, 2, 2], bf16, tag="lw3")
        for k in range(2):
            wt = np_.tile([128, 2], f32, tag="lw3stage", bufs=1)
            nc.sync.dma_start(out=wt[:, :], in_=wb_in[OFF["lw3"] + k * 128:
                                                      OFF["lw3"] + (k + 1) * 128, :2])
            nc.gpsimd.tensor_copy(lw3_dev[:, k, :], wt[:, :])
        lb1col = wp.tile([128, 4], f32, tag="lb1c")
        nc.sync.dma_start(out=lb1col[:, :],
                          in_=wb_in[OFF["lb1"], :].rearrange("(k p) -> p k", p=128))
        lb2col = wp.tile([128, 2], f32, tag="lb2c")
        nc.sync.dma_start(out=lb2col[:, :],
                          in_=wb_in[OFF["lb2"], :256].rearrange("(k p) -> p k", p=128))
        lb3col = wp.tile([2, 1], f32, tag="lb3c")
        nc.sync.dma_start(out=lb3col[:2, :],
                          in_=wb_in[OFF["lb3"]:OFF["lb3"] + 1, 0:2].rearrange("a b -> b a"))

        # xr for the current/next layer's own nodes (rotation across layers)
        xr_cur = [np2.tile([128, NBLK, H], bf16, tag="xr", name=f"xr{i}")
                  for i in range(2)]

        # ---------------- helpers ----------------
        def node_block(l, b, lhsT_fn, nk):
            """xl/xr for dst block b of layer l from feature-major lhsT chunks."""
            pxl = ps.tile([128, H], f32, tag="pnl")
            pxr = ps.tile([128, H], f32, tag="pnr")
            for k in range(nk):
                lhsT = lhsT_fn(k)
                nc.tensor.matmul(pxl[:BLK, :], lhsT, wld_t[l][:, k, :],
                                 start=(k == 0), stop=False, skip_group_check=True)
                nc.tensor.matmul(pxr[:BLK, :], lhsT, wrd_t[l][:, k, :],
                                 start=(k == 0), stop=False, skip_group_check=True)
            nc.tensor.matmul(pxl[:BLK, :], onesrow[:1, :BLK], blb_row[l][:1, :],
                             start=False, stop=True, skip_group_check=True)
            nc.tensor.matmul(pxr[:BLK, :], onesrow[:1, :BLK], brb_row[l][:1, :],
                             start=False, stop=True, skip_group_check=True)
            xl_blk = np_.tile([128, H], bf16, tag="xlblk", bufs=1)
            if l == 0:
                nc.vector.tensor_copy(xl_blk[:BLK, :], pxl[:BLK, :])
                nc.vector.tensor_copy(xr_cur[l % 2][:BLK, b, :], pxr[:BLK, :])
            else:
                nc.scalar.activation(xl_blk[:BLK, :], pxl[:BLK, :], AF.Copy)
                nc.scalar.activation(xr_cur[l % 2][:BLK, b, :], pxr[:BLK, :], AF.Copy)
            eng = nc.sync if b % 2 == 0 else nc.scalar
            eng.dma_start(out=xl_loc[l][b * BLK:(b + 1) * BLK, :],
                          in_=xl_blk[:BLK, :])

        def ag_chunk(l, ch):
            if single_core:
                nc.sync.dma_start(
                    out=xl_full[l][ch * NC * CROWS:ch * NC * CROWS + CROWS, :],
                    in_=xl_loc[l][ch * CROWS:(ch + 1) * CROWS, :])
            else:
                nc.gpsimd.collective_compute(
                    "AllGather", ALU.bypass,
                    replica_groups=[list(range(NC))],
                    ins=[xl_loc[l][ch * CROWS:(ch + 1) * CROWS, :]],
                    outs=[xl_full[l][ch * NC * CROWS:(ch + 1) * NC * CROWS, :]],
                )

        def edge_a(l, b):
            """Stage A for dst block b: gather, xr broadcast + add, prelus,
            row-reduce. Returns (xlg, esum) for stage B."""
            TB = TBs[b]
            c0 = cum[b]
            kp = KP[l]
            xlg = gp.tile([128, TBMAX, H], bf16, tag="xlg")
            nc.gpsimd.dma_gather(
                out_ap=xlg[:, :TB, :], in_ap=xl_full[l][:, :],
                idxs_ap=srcidx[:, c0 * 8:(c0 + TB) * 8],
                num_idxs=TB * 128, num_idxs_reg=TB * 128, elem_size=H,
                single_packet=False, queue_num=b % 2)
            u = ep.tile([128, TBMAX, H + 8], bf16, tag="u", bufs=2)
            for t in range(TB):
                pbc = ps2.tile([128, H], f32, tag="pbc")
                nc.tensor.matmul(pbc[:, :],
                                 e01T_sb[:BLK, (c0 + t) * 128:(c0 + t + 1) * 128],
                                 xr_cur[l % 2][:BLK, b, :],
                                 start=True, stop=True, skip_group_check=True)
                nc.vector.tensor_add(u[:, t, :H], xlg[:, t, :], pbc[:, :])
            if kp > 0:
                nc.scalar.activation(u[:, :TB, :kp], u[:, :TB, :kp],
                                     AF.Prelu, alpha=NEG)
            if kp < H:
                nc.scalar.activation(u[:, :TB, kp:H], u[:, :TB, kp:H],
                                     AF.Prelu, alpha=1.0 / NEG, scale=-1.0)
            esum = ep.tile([128, TBMAX], f32, tag="eavg", bufs=2)
            nc.vector.tensor_reduce(esum[:, :TB], u[:, :TB, :H],
                                    axis=mybir.AxisListType.X, op=ALU.add)
            return xlg, esum

        def edge_b(l, b, ctx, hT_out):
            """Stage B: exp, scatter-matmul aggregation, relu (DVE), transposes
            with 1/beta fold into hT_out."""
            TB = TBs[b]
            c0 = cum[b]
            xlg, esum = ctx
            pbuf = ep.tile([128, TBMAX], f32, tag="pbuf")
            nc.scalar.activation(pbuf[:, :TB], esum[:, :TB], AF.Exp)
            pf = ps.tile([128, H], f32, tag="pf")
            ps1 = ps.tile([128, 1], f32, tag="ps1")
            for t in range(TB):
                S = ep.tile([128, BLK], bf16, tag="S")
                nc.vector.tensor_scalar_mul(
                    S[:, :], e01_sb[:, (c0 + t) * BLK:(c0 + t + 1) * BLK],
                    pbuf[:, t:t + 1])
                nc.tensor.matmul(pf[:BLK, :], S[:, :], xlg[:, t, :],
                                 start=(t == 0), stop=(t == TB - 1),
                                 skip_group_check=True)
                nc.tensor.matmul(ps1[:BLK, :1], S[:, :], ones128[:, :1],
                                 start=(t == 0), stop=(t == TB - 1),
                                 skip_group_check=True)
            srec = ep.tile([128, 1], f32, tag="srec")
            nc.vector.reciprocal(srec[:BLK, :], ps1[:BLK, :1])
            hb = ep.tile([128, H], bf16, tag="hb", bufs=1)
            nc.vector.tensor_scalar(hb[:BLK, :], pf[:BLK, :], srec[:BLK, :], 0.0,
                                    op0=ALU.mult, op1=ALU.max)
            for kc in range(4):
                ptr = ps2.tile([128, 128], bf16, tag="ptr")
                nc.tensor.transpose(ptr[:, :BLK], hb[:BLK, ts(kc, 128)],
                                    ident[:BLK, :BLK])
                nc.vector.tensor_scalar_mul(hT_out[:, kc, b * BLK:(b + 1) * BLK],
                                            ptr[:, :BLK], recipcol[l][:, kc:kc + 1])

        # ---------------- main ----------------
        for rep in range(reps):
            # layer 0 node phase from cast x chunks
            for b in range(NBLK):
                xc = ep.tile([128, 8, BLK], bf16, tag="u", bufs=2)
                for hf in range(2):
                    xs = np_.tile([128, 4, BLK], f32, tag="xstage", bufs=1)
                    nc.sync.dma_start(
                        out=xs[:, :, :],
                        in_=x_in[hf * 512:(hf + 1) * 512,
                                 b * BLK:(b + 1) * BLK].rearrange(
                            "(k p) n -> p k n", p=128))
                    nc.vector.tensor_copy(xc[:, hf * 4:(hf + 1) * 4, :], xs[:, :, :])
                node_block(0, b, lambda k, _xc=xc: _xc[:, k, :BLK], 8)
                if (b + 1) % CBLK == 0:
                    ag_chunk(0, b // CBLK)
            hT = None
            for l in range(nlayers):
                if not nedge:
                    continue
                hT_next = np2.tile([128, 4, NLOC], bf16, tag="hT", name=f"hT{rep}_{l}")

                def finish(b, _l=l, _hT=hT_next):
                    edge_b(_l, b, ctxs[b], _hT)
                    if _l + 1 < nlayers:
                        node_block(_l + 1, b,
                                   lambda k, _h=_hT, _b=b:
                                   _h[:, k, _b * BLK:(_b + 1) * BLK], 4)
                        if (b + 1) % CBLK == 0:
                            ag_chunk(_l + 1, b // CBLK)

                ctxs = {}
                for b in range(NBLK):
                    ctxs[b] = edge_a(l, b)
                    if b > 0:
                        finish(b - 1)
                finish(NBLK - 1)
                hT = hT_next

            # ---------------- MLP head ----------------
            if hT is None:
                hT = np2.tile([128, 4, NLOC], bf16, tag="hT", name=f"hT{rep}_x")
                nc.vector.memset(hT[:, :, :], 0.0)
            jchunks = [(j0, min(128, NLOC - j0)) for j0 in range(0, NLOC, 128)]
            for j0, w in jchunks:
                h1c = np_.tile([128, 4, 128], bf16, tag="h1c", bufs=1)
                for m in range(4):
                    pm = ps2.tile([128, H], f32, tag="pbc")
                    for k in range(4):
                        nc.tensor.matmul(pm[:, :w], lw1_dev[:, k, ts(m, 128)],
                                         hT[:, k, j0:j0 + w], start=(k == 0),
                                         stop=(k == 3), skip_group_check=True)
                    nc.scalar.activation(h1c[:, m, :w], pm[:, :w], AF.Relu,
                                         bias=lb1col[:, m:m + 1])
                h2c = np_.tile([128, 2, 128], bf16, tag="h2c", bufs=1)
                for m in range(2):
                    pm = ps2.tile([128, H], f32, tag="pbc")
                    for k in range(4):
                        nc.tensor.matmul(pm[:, :w], lw2_dev[:, k, ts(m, 128)],
                                         h1c[:, k, :w], start=(k == 0),
                                         stop=(k == 3), skip_group_check=True)
                    nc.scalar.activation(h2c[:, m, :w], pm[:, :w], AF.Relu,
                                         bias=lb2col[:, m:m + 1])
                pm3 = ps2.tile([128, H], f32, tag="pbc")
                for k in range(2):
                    nc.tensor.matmul(pm3[:2, :w], lw3_dev[:, k, :],
                                     h2c[:, k, :w], start=(k == 0), stop=(k == 1),
                                     skip_group_check=True)
                logc = np_.tile([2, 128], f32, tag="logc", bufs=1)
                nc.scalar.activation(logc[:2, :w], pm3[:2, :w], AF.Identity,
                                     bias=lb3col[:2, :])
                pd = ps2.tile([128, H], f32, tag="pbc")
                nc.tensor.matmul(pd[:1, :w], sgn[:2, :], logc[:2, :w],
                                 start=True, stop=True, skip_group_check=True)
                emd = np_.tile([1, 128], f32, tag="emd", bufs=2)
                nc.scalar.activation(emd[:1, :w], pd[:1, :w], AF.Exp, scale=-1.0)
                p1c = np_.tile([1, 128], f32, tag="p1c", bufs=2)
                nc.vector.tensor_scalar_add(p1c[:1, :w], emd[:1, :w], 1.0)
                nc.vector.reciprocal(p1c[:1, :w], p1c[:1, :w])
                nc.vector.tensor_mul(emd[:1, :w], p1c[:1, :w], emd[:1, :w])
                p0c = emd
                nc.scalar.dma_start(out=logitsT_out[:, j0:j0 + w], in_=logc[:2, :w])
                nc.scalar.dma_start(out=probs0_out[:, j0:j0 + w], in_=p0c[:1, :w])
                nc.scalar.dma_start(out=probs1_out[:, j0:j0 + w], in_=p1c[:1, :w])

    nc.compile()
    return nc


_CACHE = {}
_LAST_IN_MAPS = None


def _get_program(TBs, KP):
    key = (tuple(TBs), tuple(KP))
    if key not in _CACHE:
        _CACHE[key] = _build(TBs, KP)
    return _CACHE[key]


def _run(inputs, trace=False):
    inp = {k: np.asarray(v) for k, v in inputs.items()}
    x = inp["x"].astype(np.float32)
    edge_index = inp["edge_index"].astype(np.int64)
    TBs, cores, assign = _prep_edges(edge_index)

    perms, KP = [], []
    for l in range(1, 5):
        att = inp[f"att{l}"].astype(np.float32)
        perm = np.argsort(att <= 0, kind="stable")
        perms.append(perm)
        KP.append(int((att > 0).sum()))
    wblob = _pack_wblob(inp, perms)
    ident = np.eye(128, dtype=npbf)

    ncprog = _get_program(TBs, KP)
    in_maps = []
    for c in range(NC):
        xT = np.ascontiguousarray(x[assign[c]].T)
        m = {"x": xT, "wblob": wblob, "srcidx": cores[c]["src16"],
             "e01": cores[c]["e01"], "e01T": cores[c]["e01T"], "ident": ident}
        in_maps.append(m)

    global _LAST_IN_MAPS
    _LAST_IN_MAPS = in_maps
    res = run_bass_kernel_spmd(ncprog, in_maps, list(range(NC)), trace=trace)
    logits = np.empty((N, 2), np.float32)
    probs = np.empty((N, 2), np.float32)
    for c in range(NC):
        r = res.results[c]
        logits[assign[c]] = r["logitsT"].T
        probs[assign[c], 0] = r["probs0"][0]
        probs[assign[c], 1] = r["probs1"][0]
    return (logits, probs), res


def kernel(**inputs):
    out, _ = _run(inputs, trace=False)
    return out



# revision 25
# speedup vs baseline: 1.1779x; 1.1779x over previous
"""GATv2 4-layer + MLP head on 8 Trainium2 NeuronCores (Bass/Tile), v2.

Strategy: partition destination nodes across 8 cores (1250 each, degree-
balanced into 10 blocks of 125). Per layer, per dst block:
  node : xl/xr for the block's 125 nodes via PE matmuls from the SBUF-resident
         feature-major h of the previous layer (produced by PE transposes);
         xl rows DMA to DRAM; every 2 blocks an AllGather chunk fires so the
         collective overlaps the remaining edge compute.
  edge : SWDGE-gather xl[src] rows (edge-major, bf16); xr[dst] rows broadcast
         to edges on the PE with a one-hot E01^T matmul (no second gather);
         u = xl[s]+xr[d]; e = att . leaky_relu(u) via two batched Prelu calls
         (alpha=0.2 on positive-att columns; alpha=5, scale=-1 on negative
         ones, 0.2|att| folded into the tables) + a DVE pool_avg row-reduce;
         p = exp(512*avg); aggregation via scatter matmul S = E01 * p into
         PSUM; h = relu(num/den).
Weight/bias tables are scaled on device (|att| folded in, signs via a host
column permutation; next layer unscales by 1/beta). MLP head feature-major;
softmax via exp-based sigmoid (stays in the exp ACT table set).
"""
import sys

sys.path.insert(0, "/opt/trn_rl_repo")

from contextlib import ExitStack

import numpy as np
import ml_dtypes

import concourse.bass as bass
import concourse.bacc as bacc
import concourse.tile as tile
from concourse import mybir
from concourse.bass_utils import run_bass_kernel_spmd

bf16 = mybir.dt.bfloat16
f32 = mybir.dt.float32
i16 = mybir.dt.int16
AF = mybir.ActivationFunctionType
ALU = mybir.AluOpType
ts = bass.ts
npbf = ml_dtypes.bfloat16

N, E, DIN, H = 10000, 80000, 1024, 512
NEG = 0.2
NC = 8
NLOC = N // NC          # 1250 dst nodes per core
BLK = 125               # dst nodes per block
NBLK = NLOC // BLK      # 10 blocks per core
CH = 5                  # AllGather chunks per layer
CROWS = NLOC // CH      # rows per AG chunk (250)
CBLK = NBLK // CH       # blocks per AG chunk (2)

# wbf row offsets (bf16 [WBF_ROWS, 512]): fully host-prepped device weights.
# wld/wrd rows are in (p k) order with beta = max(|att|,eps) folded in and
# the output bias riding along (blb = (bl+b)*beta, brb = (br-b)*beta).
OFF = {}
_o = 0
for _l in range(4):
    din = DIN if _l == 0 else H
    OFF[f"wl{_l}"] = _o; _o += din
    OFF[f"wr{_l}"] = _o; _o += din
OFF["lw1"] = _o; _o += H
OFF["lw2"] = _o; _o += H
OFF["lw3"] = _o; _o += 256
OFF["blb"] = _o; _o += 4
OFF["brb"] = _o; _o += 4
WBF_ROWS = _o
# wf32 [128, WF_COLS] f32 column blob: recip (1/beta) per layer, lb cols
WF_RECIP = 0            # 4 cols per layer (k p) layout
WF_LB1 = 16             # 4
WF_LB2 = 20             # 2
WF_LB3 = 22             # 1 (partitions 0:2)
WF_COLS = 23


# ---------------------------------------------------------------- host prep
def _prep_edges(edge_index):
    src = np.concatenate([edge_index[0], np.arange(N)]).astype(np.int64)
    dst = np.concatenate([edge_index[1], np.arange(N)]).astype(np.int64)
    deg = np.bincount(dst, minlength=N)
    NBUCK = NC * NBLK
    order = np.argsort(-deg, kind="stable")
    bucket_edges = np.zeros(NBUCK, np.int64)
    bucket_nodes = [[] for _ in range(NBUCK)]
    import heapq
    heap = [(0, kk) for kk in range(NBUCK)]
    heapq.heapify(heap)
    for g in order:
        while True:
            w, kk = heapq.heappop(heap)
            if len(bucket_nodes[kk]) < BLK:
                break
        bucket_nodes[kk].append(int(g))
        bucket_edges[kk] = w + int(deg[g])
        if len(bucket_nodes[kk]) < BLK:
            heapq.heappush(heap, (int(bucket_edges[kk]), kk))
    assign = [[] for _ in range(NC)]
    for c in range(NC):
        for b in range(NBLK):
            assign[c].extend(bucket_nodes[c * NBLK + b])
    assign = [np.array(a, np.int64) for a in assign]
    # chunked AG row layout: node at (core c, pos p) lands at
    # (p//CROWS)*(NC*CROWS) + c*CROWS + p%CROWS in xl_full [N, H]
    rowof = np.empty(N, np.int64)
    posof = np.empty(N, np.int64)
    coreof = np.empty(N, np.int64)
    for c in range(NC):
        p = np.arange(NLOC)
        rowof[assign[c]] = (p // CROWS) * (NC * CROWS) + c * CROWS + (p % CROWS)
        posof[assign[c]] = p
        coreof[assign[c]] = c
    percore = []
    for c in range(NC):
        sel = coreof[dst] == c
        s_, d_ = rowof[src[sel]], posof[dst[sel]]
        # within each dst block, order edges by required AG chunk of the src
        # row so leading tiles depend only on early chunks (split gathers)
        o = np.lexsort((s_ // (NC * CROWS), d_ // BLK))
        s_, d_ = s_[o], d_[o]
        blocks = []
        for bb in range(NBLK):
            m = (d_ // BLK) == bb
            blocks.append((s_[m], d_[m] - bb * BLK))
        percore.append(blocks)
    TBs = tuple(max(max(-(-len(percore[c][b][0]) // 128), 1) for c in range(NC))
                for b in range(NBLK))
    cum = np.concatenate([[0], np.cumsum(TBs)]).astype(int)
    NT = int(cum[-1])
    EPAD = NT * 128
    # per block: shared (min-over-cores) tile counts whose srcs all sit in
    # AG chunks <= ch; GB[b] is a nondecreasing list of 5 tile boundaries
    GB = []
    for b in range(NBLK):
        gb = []
        for ch in range(CH):
            lim = (ch + 1) * NC * CROWS
            cnt = min(int((percore[c][b][0] < lim).sum()) // 128
                      for c in range(NC))
            gb.append(cnt)
        gb[-1] = TBs[b]
        GB.append(tuple(gb))
    cores = []
    for c in range(NC):
        src16 = np.zeros(EPAD, np.int16)
        e01 = np.zeros((128, NT * BLK), npbf)
        e01T = np.zeros((128, NT * 128), npbf)
        for b in range(NBLK):
            s, d = percore[c][b]
            n = len(s)
            base = int(cum[b]) * 128
            src16[base:base + n] = s
            tt = int(cum[b]) + np.arange(n) // 128
            pp = np.arange(n) % 128
            e01[pp, tt * BLK + d] = 1.0
            e01T[d, tt * 128 + pp] = 1.0
        w = src16.reshape(-1, 16).T.copy()
        cores.append(dict(src16=np.tile(w, (8, 1)).copy(),
                          e01=np.ascontiguousarray(e01),
                          e01T=np.ascontiguousarray(e01T)))
    return (TBs, tuple(GB)), cores, assign


def _pack_weights(inp, perms):
    """Host-side full weight prep: beta-scaled bf16 blob + small f32 cols."""
    def pk(W):
        nk = W.shape[0] // 128
        return W.reshape(nk, 128, W.shape[1]).transpose(1, 0, 2).reshape(
            nk * 128, W.shape[1])

    wbf = np.zeros((WBF_ROWS, H), npbf)
    wf = np.zeros((128, WF_COLS), np.float32)
    for l in range(4):
        rowp = perms[l - 1] if l > 0 else None
        att = np.asarray(inp[f"att{l + 1}"], np.float32)[perms[l]]
        beta = np.maximum(np.abs(att), 1e-30)
        bb = np.asarray(inp[f"b{l + 1}"], np.float32)[perms[l]]
        bl = np.asarray(inp[f"bl{l + 1}"], np.float32)[perms[l]]
        br = np.asarray(inp[f"br{l + 1}"], np.float32)[perms[l]]
        for nm, bias in (("wl", bl + bb), ("wr", br - bb)):
            W = np.asarray(inp[f"{nm}{l + 1}"], np.float32)
            if rowp is not None:
                W = W[rowp, :]
            W = W[:, perms[l]] * beta[None, :]
            wbf[OFF[f"{nm}{l}"]:OFF[f"{nm}{l}"] + W.shape[0], :] = pk(W)
            off = OFF["blb" if nm == "wl" else "brb"] + l
            wbf[off, :] = bias * beta
        wf[:, WF_RECIP + 4 * l:WF_RECIP + 4 * l + 4] = (
            1.0 / beta).reshape(4, 128).T
    wbf[OFF["lw1"]:OFF["lw1"] + H, :] = pk(
        np.asarray(inp["lw1"], np.float32)[perms[3], :])
    wbf[OFF["lw2"]:OFF["lw2"] + H, :256] = pk(np.asarray(inp["lw2"], np.float32))
    wbf[OFF["lw3"]:OFF["lw3"] + 256, :2] = pk(np.asarray(inp["lw3"], np.float32))
    wf[:, WF_LB1:WF_LB1 + 4] = np.asarray(inp["lb1"], np.float32).reshape(4, 128).T
    wf[:, WF_LB2:WF_LB2 + 2] = np.asarray(inp["lb2"], np.float32).reshape(2, 128).T
    wf[0:2, WF_LB3] = np.asarray(inp["lb3"], np.float32)
    return wbf, wf


# -------------------------------------------------------------- bass program
def _build(TB_info, KP, single_core=False, nlayers=4, nedge=True, reps=1):
    TBs, GB = TB_info
    TBs = tuple(TBs)
    TBMAX = max(TBs)
    cum = [0]
    for t in TBs:
        cum.append(cum[-1] + t)
    NT = cum[-1]
    nc = bacc.Bacc("TRN2", num_swdge_queues=2)
    P = nc.declare_dram_parameter
    x_in = P("x", [DIN, NLOC], bf16, isOutput=False)
    wb_in = P("wbf", [WBF_ROWS, H], bf16, isOutput=False)
    wf_in = P("wf32", [128, WF_COLS], f32, isOutput=False)
    srcidx_in = P("srcidx", [128, NT * 8], i16, isOutput=False)
    e01_in = P("e01", [128, NT * BLK], bf16, isOutput=False)
    e01T_in = P("e01T", [128, NT * 128], bf16, isOutput=False)
    ident_in = P("ident", [128, 128], bf16, isOutput=False)
    logitsT_out = P("logitsT", [2, NLOC], f32, isOutput=True)
    probs0_out = P("probs0", [1, NLOC], f32, isOutput=True)
    probs1_out = P("probs1", [1, NLOC], f32, isOutput=True)

    xl_loc = [nc.dram_tensor(f"xlloc{l}", [NLOC, H], bf16) for l in range(4)]
    xl_full = [nc.dram_tensor(f"xlfull{l}", [N, H], bf16, addr_space="Shared")
               for l in range(4)]

    with tile.TileContext(nc) as tc, ExitStack() as ctx:
        wp = ctx.enter_context(tc.tile_pool(name="wp", bufs=1))
        np_ = ctx.enter_context(tc.tile_pool(name="np", bufs=2))
        np2 = ctx.enter_context(tc.tile_pool(name="np2", bufs=2))
        ep = ctx.enter_context(tc.tile_pool(name="ep", bufs=2))
        gp = ctx.enter_context(tc.tile_pool(name="gp", bufs=2))
        ps = ctx.enter_context(tc.tile_pool(name="ps", bufs=1, space="PSUM"))
        ps2 = ctx.enter_context(tc.tile_pool(name="ps2", bufs=2, space="PSUM"))

        # ---------------- constants ----------------
        ones128 = wp.tile([128, 1], bf16, tag="ones128")
        nc.vector.memset(ones128[:, :], 1.0)
        onesrow = wp.tile([1, 128], bf16, tag="onesrow")
        nc.vector.memset(onesrow[:1, :], 1.0)
        sgn = wp.tile([2, 1], f32, tag="sgn")
        nc.vector.memset(sgn[:2, :], 1.0)
        nc.vector.tensor_scalar_mul(sgn[0:1, :], sgn[0:1, :], -1.0)
        ident = wp.tile([128, 128], bf16, tag="ident")
        nc.scalar.dma_start(out=ident[:, :], in_=ident_in[:, :])

        # ---------------- weights (all host-prepped, straight DMA loads) -----
        # layer-0 weights on the sync queue (first need), rest on scalar
        wld_t, wrd_t, blb_row, brb_row, recipcol = [], [], [], [], []
        for l in range(4):
            din = DIN if l == 0 else H
            nk0 = din // 128
            weng = nc.sync if l == 0 else nc.scalar
            wld = wp.tile([128, nk0, H], bf16, tag=f"wld{l}")
            wrd = wp.tile([128, nk0, H], bf16, tag=f"wrd{l}")
            wld_t.append(wld)
            wrd_t.append(wrd)
            for W_off, wdev in ((OFF[f"wl{l}"], wld), (OFF[f"wr{l}"], wrd)):
                weng.dma_start(out=wdev[:, :, :],
                               in_=wb_in[W_off:W_off + nk0 * 128, :].rearrange(
                                   "(p k) h -> p k h", p=128))
            t = wp.tile([1, H], bf16, tag=f"blb{l}")
            weng.dma_start(out=t[:1, :], in_=wb_in[OFF["blb"] + l:OFF["blb"] + l + 1, :])
            blb_row.append(t)
            t = wp.tile([1, H], bf16, tag=f"brb{l}")
            weng.dma_start(out=t[:1, :], in_=wb_in[OFF["brb"] + l:OFF["brb"] + l + 1, :])
            brb_row.append(t)
        wfcols = wp.tile([128, WF_COLS], f32, tag="wfcols")
        nc.sync.dma_start(out=wfcols[:, :], in_=wf_in[:, :])
        recipcol = [wfcols[:, WF_RECIP + 4 * l:WF_RECIP + 4 * l + 4]
                    for l in range(4)]
        lb1col = wfcols[:, WF_LB1:WF_LB1 + 4]
        lb2col = wfcols[:, WF_LB2:WF_LB2 + 2]
        lb3col = wfcols[:, WF_LB3:WF_LB3 + 1]

        # ---------------- MLP weights ----------------
        lw1_dev = wp.tile([128, 4, H], bf16, tag="lw1")
        nc.scalar.dma_start(out=lw1_dev[:, :, :],
                            in_=wb_in[OFF["lw1"]:OFF["lw1"] + H, :].rearrange(
                                "(p k) h -> p k h", p=128))
        lw2_dev = wp.tile([128, 4, 256], bf16, tag="lw2")
        nc.scalar.dma_start(out=lw2_dev[:, :, :],
                            in_=wb_in[OFF["lw2"]:OFF["lw2"] + H, :256].rearrange(
                                "(p k) h -> p k h", p=128))
        lw3_dev = wp.tile([128, 2, 2], bf16, tag="lw3")
        nc.scalar.dma_start(out=lw3_dev[:, :, :],
                            in_=wb_in[OFF["lw3"]:OFF["lw3"] + 256, :2].rearrange(
                                "(p k) h -> p k h", p=128))

        # edge-phase constants: big loads deferred here so the sync queue
        # serves layer-0 weights/x first (edge_a needs these only after the
        # first node blocks).
        srcidx = wp.tile([128, NT * 8], i16, tag="srcidx")
        nc.scalar.dma_start(out=srcidx[:, :], in_=srcidx_in[:, :])
        e01T_sb = wp.tile([128, NT * 128], bf16, tag="e01T")
        nc.scalar.dma_start(out=e01T_sb[:, :], in_=e01T_in[:, :])
        e01_sb = wp.tile([128, NT * BLK], bf16, tag="e01")
        nc.scalar.dma_start(out=e01_sb[:, :], in_=e01_in[:, :])

        # xr for the current/next layer's own nodes (rotation across layers)
        xr_cur = [np2.tile([128, NBLK, H], bf16, tag="xr", name=f"xr{i}")
                  for i in range(2)]

        # ---------------- helpers ----------------
        def node_block(l, b, lhsT_fn, nk):
            """xl/xr for dst block b of layer l from feature-major lhsT chunks."""
            pxl = ps.tile([128, H], f32, tag="pnl")
            pxr = ps.tile([128, H], f32, tag="pnr")
            for k in range(nk):
                lhsT = lhsT_fn(k)
                nc.tensor.matmul(pxl[:BLK, :], lhsT, wld_t[l][:, k, :],
                                 start=(k == 0), stop=False, skip_group_check=True)
                nc.tensor.matmul(pxr[:BLK, :], lhsT, wrd_t[l][:, k, :],
                                 start=(k == 0), stop=False, skip_group_check=True)
            nc.tensor.matmul(pxl[:BLK, :], onesrow[:1, :BLK], blb_row[l][:1, :],
                             start=False, stop=True, skip_group_check=True)
            nc.tensor.matmul(pxr[:BLK, :], onesrow[:1, :BLK], brb_row[l][:1, :],
                             start=False, stop=True, skip_group_check=True)
            xl_blk = np_.tile([128, H], bf16, tag="xlblk", bufs=2)
            nc.vector.tensor_copy(xl_blk[:BLK, :], pxl[:BLK, :])
            nc.vector.tensor_copy(xr_cur[l % 2][:BLK, b, :], pxr[:BLK, :])
            eng = nc.sync if b % 2 == 0 else nc.scalar
            eng.dma_start(out=xl_loc[l][b * BLK:(b + 1) * BLK, :],
                          in_=xl_blk[:BLK, :])

        def ag_chunk(l, ch):
            if single_core:
                for cc in range(NC):
                    eng = nc.sync if cc % 2 == 0 else nc.scalar
                    eng.dma_start(
                        out=xl_full[l][ch * NC * CROWS + cc * CROWS:
                                       ch * NC * CROWS + (cc + 1) * CROWS, :],
                        in_=xl_loc[l][ch * CROWS:(ch + 1) * CROWS, :])
            else:
                nc.gpsimd.collective_compute(
                    "AllGather", ALU.bypass,
                    replica_groups=[list(range(NC))],
                    ins=[xl_loc[l][ch * CROWS:(ch + 1) * CROWS, :]],
                    outs=[xl_full[l][ch * NC * CROWS:(ch + 1) * NC * CROWS, :]],
                )

        def edge_a(l, b):
            """Stage A for dst block b: split gathers (leading tiles depend
            only on early AG chunks); per tile, accumulate u = xr[dst]
            (one-hot matmul) + xlg (ident matmul) in PSUM, one full-width
            Prelu with fused accum row-sum (S_all) + DVE reduce of the
            neg-att slab; e = S_all - 2*S_neg. Returns (xlg, esum)."""
            TB = TBs[b]
            c0 = cum[b]
            kp = KP[l]
            xlg = gp.tile([128, TBMAX, H], bf16, tag="xlg")
            # split gathers by required AG chunk: fine (5-way) for layer 0
            # where the serial AG chain is exposed, coarse (early/late)
            # after; each dma_gather costs ~1us of Q7 descgen.
            if l == 0:
                splits = [(GB[b][ch], ch) for ch in range(CH)]
            else:
                splits = [(GB[b][CH - 2], CH - 2), (GB[b][CH - 1], CH - 1)]
            t0 = 0
            for t1, ch in splits:
                if t1 <= t0:
                    continue
                rows = (ch + 1) * NC * CROWS
                nc.gpsimd.dma_gather(
                    out_ap=xlg[:, t0:t1, :], in_ap=xl_full[l][0:rows, :],
                    idxs_ap=srcidx[:, (c0 + t0) * 8:(c0 + t1) * 8],
                    num_idxs=(t1 - t0) * 128, num_idxs_reg=(t1 - t0) * 128,
                    elem_size=H, single_packet=False, queue_num=b % 2)
                t0 = t1
            es_all = ep.tile([128, TBMAX], f32, tag="esall", bufs=2)
            es_neg = ep.tile([128, TBMAX], f32, tag="esneg", bufs=2)
            for t in range(TB):
                pbc = ps2.tile([128, H], f32, tag="pbc")
                nc.tensor.matmul(pbc[:, :],
                                 e01T_sb[:BLK, (c0 + t) * 128:(c0 + t + 1) * 128],
                                 xr_cur[l % 2][:BLK, b, :],
                                 start=True, stop=False, skip_group_check=True)
                nc.tensor.matmul(pbc[:, :], ident[:, :], xlg[:, t, :],
                                 start=False, stop=True, skip_group_check=True)
                wscr = ep.tile([128, H], bf16, tag="wscr", bufs=2)
                nc.scalar.activation(wscr[:, :], pbc[:, :], AF.Prelu, alpha=NEG,
                                     accum_out=es_all[:, t:t + 1])
                if kp < H:
                    nc.vector.tensor_reduce(es_neg[:, t:t + 1], wscr[:, kp:H],
                                            axis=mybir.AxisListType.X, op=ALU.add)
            esum = ep.tile([128, TBMAX], f32, tag="eavg", bufs=2)
            if kp == H:
                esum = es_all
            else:
                nc.vector.tensor_scalar_mul(es_neg[:, :TB], es_neg[:, :TB], -2.0)
                nc.vector.tensor_add(esum[:, :TB], es_all[:, :TB], es_neg[:, :TB])
            return xlg, esum

        def edge_b(l, b, ctx, hT_out):
            """Stage B: exp, scatter-matmul aggregation, relu (DVE), transposes
            with 1/beta fold into hT_out."""
            TB = TBs[b]
            c0 = cum[b]
            xlg, esum = ctx
            pbuf = ep.tile([128, TBMAX], f32, tag="pbuf")
            nc.scalar.activation(pbuf[:, :TB], esum[:, :TB], AF.Exp)
            pf = ps.tile([128, H], f32, tag="pf")
            ps1 = ps.tile([128, 1], f32, tag="ps1")
            for t in range(TB):
                S = ep.tile([128, BLK], bf16, tag="S")
                nc.vector.tensor_scalar_mul(
                    S[:, :], e01_sb[:, (c0 + t) * BLK:(c0 + t + 1) * BLK],
                    pbuf[:, t:t + 1])
                nc.tensor.matmul(pf[:BLK, :], S[:, :], xlg[:, t, :],
                                 start=(t == 0), stop=(t == TB - 1),
                                 skip_group_check=True)
                nc.tensor.matmul(ps1[:BLK, :1], S[:, :], ones128[:, :1],
                                 start=(t == 0), stop=(t == TB - 1),
                                 skip_group_check=True)
            srec = ep.tile([128, 1], f32, tag="srec")
            nc.vector.reciprocal(srec[:BLK, :], ps1[:BLK, :1])
            hb = ep.tile([128, H], bf16, tag="hb", bufs=1)
            nc.vector.tensor_scalar(hb[:BLK, :], pf[:BLK, :], srec[:BLK, :], 0.0,
                                    op0=ALU.mult, op1=ALU.max)
            for kc in range(4):
                ptr = ps2.tile([128, 128], bf16, tag="ptr")
                nc.tensor.transpose(ptr[:, :BLK], hb[:BLK, ts(kc, 128)],
                                    ident[:BLK, :BLK])
                nc.vector.tensor_scalar_mul(hT_out[:, kc, b * BLK:(b + 1) * BLK],
                                            ptr[:, :BLK], recipcol[l][:, kc:kc + 1])

        # ---------------- MLP head (per 128-node chunk) ----------------
        jchunks = [(j0, min(128, NLOC - j0)) for j0 in range(0, NLOC, 128)]
        mlp_ready_at = {}
        for _j, (_j0, _w) in enumerate(jchunks):
            mlp_ready_at.setdefault((_j0 + _w - 1) // BLK, []).append(_j)

        def mlp_chunk(jidx, hT):
            j0, w = jchunks[jidx]
            h1c = np_.tile([128, 4, 128], bf16, tag="h1c", bufs=2)
            for m in range(4):
                pm = ps2.tile([128, H], f32, tag="pbc")
                for k in range(4):
                    nc.tensor.matmul(pm[:, :w], lw1_dev[:, k, ts(m, 128)],
                                     hT[:, k, j0:j0 + w], start=(k == 0),
                                     stop=(k == 3), skip_group_check=True)
                nc.scalar.activation(h1c[:, m, :w], pm[:, :w], AF.Relu,
                                     bias=lb1col[:, m:m + 1])
            h2c = np_.tile([128, 2, 128], bf16, tag="h2c", bufs=2)
            for m in range(2):
                pm = ps2.tile([128, H], f32, tag="pbc")
                for k in range(4):
                    nc.tensor.matmul(pm[:, :w], lw2_dev[:, k, ts(m, 128)],
                                     h1c[:, k, :w], start=(k == 0),
                                     stop=(k == 3), skip_group_check=True)
                nc.scalar.activation(h2c[:, m, :w], pm[:, :w], AF.Relu,
                                     bias=lb2col[:, m:m + 1])
            pm3 = ps2.tile([128, H], f32, tag="pbc")
            for k in range(2):
                nc.tensor.matmul(pm3[:2, :w], lw3_dev[:, k, :],
                                 h2c[:, k, :w], start=(k == 0), stop=(k == 1),
                                 skip_group_check=True)
            logc = np_.tile([2, 128], f32, tag="logc", bufs=2)
            nc.scalar.activation(logc[:2, :w], pm3[:2, :w], AF.Identity,
                                 bias=lb3col[:2, :])
            pd = ps2.tile([128, H], f32, tag="pbc")
            nc.tensor.matmul(pd[:1, :w], sgn[:2, :], logc[:2, :w],
                             start=True, stop=True, skip_group_check=True)
            emd = np_.tile([1, 128], f32, tag="emd", bufs=2)
            nc.scalar.activation(emd[:1, :w], pd[:1, :w], AF.Exp, scale=-1.0)
            p1c = np_.tile([1, 128], f32, tag="p1c", bufs=2)
            nc.vector.tensor_scalar_add(p1c[:1, :w], emd[:1, :w], 1.0)
            nc.vector.reciprocal(p1c[:1, :w], p1c[:1, :w])
            nc.vector.tensor_mul(emd[:1, :w], p1c[:1, :w], emd[:1, :w])
            p0c = emd
            nc.scalar.dma_start(out=logitsT_out[:, j0:j0 + w], in_=logc[:2, :w])
            nc.scalar.dma_start(out=probs0_out[:, j0:j0 + w], in_=p0c[:1, :w])
            nc.scalar.dma_start(out=probs1_out[:, j0:j0 + w], in_=p1c[:1, :w])

        # ---------------- main ----------------
        for rep in range(reps):
            # layer 0 node phase straight from bf16 x chunks
            for b in range(NBLK):
                xc = ep.tile([128, 8, BLK], bf16, tag="xc", bufs=2)
                nc.sync.dma_start(
                    out=xc[:, :, :],
                    in_=x_in[:, b * BLK:(b + 1) * BLK].rearrange(
                        "(k p) n -> p k n", p=128))
                node_block(0, b, lambda k, _xc=xc: _xc[:, k, :BLK], 8)
                if (b + 1) % CBLK == 0:
                    ag_chunk(0, b // CBLK)
            hT = None
            for l in range(nlayers):
                if not nedge:
                    continue
                hT_next = np2.tile([128, 4, NLOC], bf16, tag="hT", name=f"hT{rep}_{l}")

                def finish(b, _l=l, _hT=hT_next):
                    edge_b(_l, b, ctxs[b], _hT)
                    if _l + 1 < nlayers:
                        node_block(_l + 1, b,
                                   lambda k, _h=_hT, _b=b:
                                   _h[:, k, _b * BLK:(_b + 1) * BLK], 4)
                        if (b + 1) % CBLK == 0:
                            ag_chunk(_l + 1, b // CBLK)
                    else:
                        for j in mlp_ready_at.get(b, []):
                            mlp_chunk(j, _hT)

                ctxs = {}
                for b in range(NBLK):
                    ctxs[b] = edge_a(l, b)
                    if b > 0:
                        finish(b - 1)
                finish(NBLK - 1)
                hT = hT_next

            # ---------------- MLP head (fallback when no edge layers ran) ----
            if hT is None:
                hT = np2.tile([128, 4, NLOC], bf16, tag="hT", name=f"hT{rep}_x")
                nc.vector.memset(hT[:, :, :], 0.0)
                for j in range(len(jchunks)):
                    mlp_chunk(j, hT)

    nc.compile()
    return nc


_CACHE = {}
_LAST_IN_MAPS = None


def _get_program(TBs, KP):
    key = (tuple(TBs), tuple(KP))
    if key not in _CACHE:
        _CACHE[key] = _build(TBs, KP)
    return _CACHE[key]


def _run(inputs, trace=False):
    inp = {k: np.asarray(v) for k, v in inputs.items()}
    x = inp["x"].astype(np.float32)
    edge_index = inp["edge_index"].astype(np.int64)
    TBs, cores, assign = _prep_edges(edge_index)

    perms, KP = [], []
    for l in range(1, 5):
        att = inp[f"att{l}"].astype(np.float32)
        perm = np.argsort(att <= 0, kind="stable")
        perms.append(perm)
        KP.append(int((att > 0).sum()))
    wbf, wf = _pack_weights(inp, perms)
    ident = np.eye(128, dtype=npbf)

    ncprog = _get_program(TBs, KP)
    in_maps = []
    for c in range(NC):
        xT = np.ascontiguousarray(x[assign[c]].T.astype(npbf))
        m = {"x": xT, "wbf": wbf, "wf32": wf, "srcidx": cores[c]["src16"],
             "e01": cores[c]["e01"], "e01T": cores[c]["e01T"], "ident": ident}
        in_maps.append(m)

    global _LAST_IN_MAPS
    _LAST_IN_MAPS = in_maps
    res = run_bass_kernel_spmd(ncprog, in_maps, list(range(NC)), trace=trace)
    logits = np.empty((N, 2), np.float32)
    probs = np.empty((N, 2), np.float32)
    for c in range(NC):
        r = res.results[c]
        logits[assign[c]] = r["logitsT"].T
        probs[assign[c], 0] = r["probs0"][0]
        probs[assign[c], 1] = r["probs1"][0]
    return (logits, probs), res


def kernel(**inputs):
    out, _ = _run(inputs, trace=False)
    return out



# revision 32
# speedup vs baseline: 1.5463x; 1.3128x over previous
"""GATv2 4-layer + MLP head on 8 Trainium2 NeuronCores (Bass/Tile), v2.

Strategy: partition destination nodes across 8 cores (1250 each, degree-
balanced into 10 blocks of 125). Per layer, per dst block:
  node : xl/xr for the block's 125 nodes via PE matmuls from the SBUF-resident
         feature-major h of the previous layer (produced by PE transposes);
         xl rows DMA to DRAM; every 2 blocks an AllGather chunk fires so the
         collective overlaps the remaining edge compute.
  edge : SWDGE-gather xl[src] rows (edge-major, bf16); xr[dst] rows broadcast
         to edges on the PE with a one-hot E01^T matmul (no second gather);
         u = xl[s]+xr[d]; e = att . leaky_relu(u) via two batched Prelu calls
         (alpha=0.2 on positive-att columns; alpha=5, scale=-1 on negative
         ones, 0.2|att| folded into the tables) + a DVE pool_avg row-reduce;
         p = exp(512*avg); aggregation via scatter matmul S = E01 * p into
         PSUM; h = relu(num/den).
Weight/bias tables are scaled on device (|att| folded in, signs via a host
column permutation; next layer unscales by 1/beta). MLP head feature-major;
softmax via exp-based sigmoid (stays in the exp ACT table set).
"""
import sys

sys.path.insert(0, "/opt/trn_rl_repo")

from contextlib import ExitStack

import numpy as np
import ml_dtypes

import concourse.bass as bass
import concourse.bacc as bacc
import concourse.tile as tile
from concourse import mybir
from concourse.bass_utils import run_bass_kernel_spmd

bf16 = mybir.dt.bfloat16
f32 = mybir.dt.float32
i16 = mybir.dt.int16
AF = mybir.ActivationFunctionType
ALU = mybir.AluOpType
ts = bass.ts
npbf = ml_dtypes.bfloat16

N, E, DIN, H = 10000, 80000, 1024, 512
NEG = 0.2
NC = 8
NLOC = N // NC          # 1250 dst nodes per core
BLK = 125               # dst nodes per block
NBLK = NLOC // BLK      # 10 blocks per core
CH = 5                  # AllGather chunks per layer
CROWS = NLOC // CH      # rows per AG chunk (250)
CBLK = NBLK // CH       # blocks per AG chunk (2)

# wbf row offsets (bf16 [WBF_ROWS, 512]): fully host-prepped device weights.
# wld/wrd rows are in (p k) order with beta = max(|att|,eps) folded in and
# the output bias riding along (blb = (bl+b)*beta, brb = (br-b)*beta).
OFF = {}
_o = 0
for _l in range(4):
    din = DIN if _l == 0 else H
    OFF[f"wl{_l}"] = _o; _o += din
    OFF[f"wr{_l}"] = _o; _o += din
OFF["lw1"] = _o; _o += H
OFF["lw2"] = _o; _o += H
OFF["lw3"] = _o; _o += 256
OFF["blb"] = _o; _o += 4
OFF["brb"] = _o; _o += 4
WBF_ROWS = _o
# wf32 [128, WF_COLS] f32 column blob: recip (1/beta) per layer, lb cols
WF_RECIP = 0            # 4 cols per layer (k p) layout
WF_LB1 = 16             # 4
WF_LB2 = 20             # 2
WF_LB3 = 22             # 1 (partitions 0:2)
WF_COLS = 23


# ---------------------------------------------------------------- host prep
def _prep_edges(edge_index):
    src = np.concatenate([edge_index[0], np.arange(N)]).astype(np.int64)
    dst = np.concatenate([edge_index[1], np.arange(N)]).astype(np.int64)
    deg = np.bincount(dst, minlength=N)
    NBUCK = NC * NBLK
    order = np.argsort(-deg, kind="stable")
    bucket_edges = np.zeros(NBUCK, np.int64)
    bucket_nodes = [[] for _ in range(NBUCK)]
    import heapq
    heap = [(0, kk) for kk in range(NBUCK)]
    heapq.heapify(heap)
    for g in order:
        while True:
            w, kk = heapq.heappop(heap)
            if len(bucket_nodes[kk]) < BLK:
                break
        bucket_nodes[kk].append(int(g))
        bucket_edges[kk] = w + int(deg[g])
        if len(bucket_nodes[kk]) < BLK:
            heapq.heappush(heap, (int(bucket_edges[kk]), kk))
    assign = [[] for _ in range(NC)]
    for c in range(NC):
        for b in range(NBLK):
            assign[c].extend(bucket_nodes[c * NBLK + b])
    assign = [np.array(a, np.int64) for a in assign]
    # chunked AG row layout: node at (core c, pos p) lands at
    # (p//CROWS)*(NC*CROWS) + c*CROWS + p%CROWS in xl_full [N, H]
    rowof = np.empty(N, np.int64)
    posof = np.empty(N, np.int64)
    coreof = np.empty(N, np.int64)
    for c in range(NC):
        p = np.arange(NLOC)
        rowof[assign[c]] = (p // CROWS) * (NC * CROWS) + c * CROWS + (p % CROWS)
        posof[assign[c]] = p
        coreof[assign[c]] = c
    percore = []
    for c in range(NC):
        sel = coreof[dst] == c
        s_, d_ = rowof[src[sel]], posof[dst[sel]]
        # within each dst block, order edges by required AG chunk of the src
        # row so leading tiles depend only on early chunks (split gathers)
        o = np.lexsort((s_ // (NC * CROWS), d_ // BLK))
        s_, d_ = s_[o], d_[o]
        blocks = []
        for bb in range(NBLK):
            m = (d_ // BLK) == bb
            blocks.append((s_[m], d_[m] - bb * BLK))
        percore.append(blocks)
    TBs = tuple(max(max(-(-len(percore[c][b][0]) // 128), 1) for c in range(NC))
                for b in range(NBLK))
    cum = np.concatenate([[0], np.cumsum(TBs)]).astype(int)
    NT = int(cum[-1])
    EPAD = NT * 128
    # per block: shared (min-over-cores) tile counts whose srcs all sit in
    # AG chunks <= ch; GB[b] is a nondecreasing list of 5 tile boundaries
    GB = []
    for b in range(NBLK):
        gb = []
        for ch in range(CH):
            lim = (ch + 1) * NC * CROWS
            cnt = min(int((percore[c][b][0] < lim).sum()) // 128
                      for c in range(NC))
            gb.append(cnt)
        gb[-1] = TBs[b]
        GB.append(tuple(gb))
    cores = []
    for c in range(NC):
        src16 = np.zeros(EPAD, np.int16)
        e01 = np.zeros((128, NT * BLK), npbf)
        e01T = np.zeros((128, NT * 128), npbf)
        for b in range(NBLK):
            s, d = percore[c][b]
            n = len(s)
            base = int(cum[b]) * 128
            src16[base:base + n] = s
            tt = int(cum[b]) + np.arange(n) // 128
            pp = np.arange(n) % 128
            e01[pp, tt * BLK + d] = 1.0
            e01T[d, tt * 128 + pp] = 1.0
        w = src16.reshape(-1, 16).T.copy()
        cores.append(dict(src16=np.tile(w, (8, 1)).copy(),
                          e01=np.ascontiguousarray(e01),
                          e01T=np.ascontiguousarray(e01T)))
    return (TBs, tuple(GB)), cores, assign


def _pack_weights(inp, perms):
    """Host-side full weight prep: beta-scaled bf16 blob + small f32 cols."""
    def pk(W):
        nk = W.shape[0] // 128
        return W.reshape(nk, 128, W.shape[1]).transpose(1, 0, 2).reshape(
            nk * 128, W.shape[1])

    wbf = np.zeros((WBF_ROWS, H), npbf)
    wf = np.zeros((128, WF_COLS), np.float32)
    for l in range(4):
        rowp = perms[l - 1] if l > 0 else None
        att = np.asarray(inp[f"att{l + 1}"], np.float32)[perms[l]]
        beta = np.maximum(np.abs(att), 1e-30)
        bb = np.asarray(inp[f"b{l + 1}"], np.float32)[perms[l]]
        bl = np.asarray(inp[f"bl{l + 1}"], np.float32)[perms[l]]
        br = np.asarray(inp[f"br{l + 1}"], np.float32)[perms[l]]
        for nm, bias in (("wl", bl + bb), ("wr", br - bb)):
            W = np.asarray(inp[f"{nm}{l + 1}"], np.float32)
            if rowp is not None:
                W = W[rowp, :]
            W = W[:, perms[l]] * beta[None, :]
            wbf[OFF[f"{nm}{l}"]:OFF[f"{nm}{l}"] + W.shape[0], :] = pk(W)
            off = OFF["blb" if nm == "wl" else "brb"] + l
            wbf[off, :] = bias * beta
        wf[:, WF_RECIP + 4 * l:WF_RECIP + 4 * l + 4] = (
            1.0 / beta).reshape(4, 128).T
    wbf[OFF["lw1"]:OFF["lw1"] + H, :] = pk(
        np.asarray(inp["lw1"], np.float32)[perms[3], :])
    wbf[OFF["lw2"]:OFF["lw2"] + H, :256] = pk(np.asarray(inp["lw2"], np.float32))
    wbf[OFF["lw3"]:OFF["lw3"] + 256, :2] = pk(np.asarray(inp["lw3"], np.float32))
    wf[:, WF_LB1:WF_LB1 + 4] = np.asarray(inp["lb1"], np.float32).reshape(4, 128).T
    wf[:, WF_LB2:WF_LB2 + 2] = np.asarray(inp["lb2"], np.float32).reshape(2, 128).T
    wf[0:2, WF_LB3] = np.asarray(inp["lb3"], np.float32)
    return wbf, wf


# -------------------------------------------------------------- bass program
def _build(TB_info, KP, single_core=False, nlayers=4, nedge=True, reps=1):
    TBs, GB = TB_info
    TBs = tuple(TBs)
    TBMAX = max(TBs)
    cum = [0]
    for t in TBs:
        cum.append(cum[-1] + t)
    NT = cum[-1]
    nc = bacc.Bacc("TRN2", num_swdge_queues=2)
    P = nc.declare_dram_parameter
    xf_in = P("xf", [DIN, N], bf16, isOutput=False)
    x_in = P("x", [DIN, NLOC], bf16, isOutput=False)
    wb_in = P("wbf", [WBF_ROWS, H], bf16, isOutput=False)
    wf_in = P("wf32", [128, WF_COLS], f32, isOutput=False)
    srcidx_in = P("srcidx", [128, NT * 8], i16, isOutput=False)
    e01_in = P("e01", [128, NT * BLK], bf16, isOutput=False)
    e01T_in = P("e01T", [128, NT * 128], bf16, isOutput=False)
    ident_in = P("ident", [128, 128], bf16, isOutput=False)
    logitsT_out = P("logitsT", [2, NLOC], f32, isOutput=True)
    probs0_out = P("probs0", [1, NLOC], f32, isOutput=True)
    probs1_out = P("probs1", [1, NLOC], f32, isOutput=True)

    xl_loc = [nc.dram_tensor(f"xlloc{l}", [NLOC, H], bf16) for l in range(4)]
    xl_full = [nc.dram_tensor(f"xlfull{l}", [N, H], bf16, addr_space="Shared")
               for l in range(4)]

    with tile.TileContext(nc) as tc, ExitStack() as ctx:
        wp = ctx.enter_context(tc.tile_pool(name="wp", bufs=1))
        np_ = ctx.enter_context(tc.tile_pool(name="np", bufs=2))
        np2 = ctx.enter_context(tc.tile_pool(name="np2", bufs=2))
        ep = ctx.enter_context(tc.tile_pool(name="ep", bufs=2))
        gp = ctx.enter_context(tc.tile_pool(name="gp", bufs=2))
        ps = ctx.enter_context(tc.tile_pool(name="ps", bufs=1, space="PSUM"))
        ps2 = ctx.enter_context(tc.tile_pool(name="ps2", bufs=2, space="PSUM"))

        # ---------------- constants ----------------
        ones128 = wp.tile([128, 1], bf16, tag="ones128")
        nc.vector.memset(ones128[:, :], 1.0)
        onesrow = wp.tile([1, 128], bf16, tag="onesrow")
        nc.vector.memset(onesrow[:1, :], 1.0)
        sgn = wp.tile([2, 1], f32, tag="sgn")
        nc.vector.memset(sgn[:2, :], 1.0)
        nc.vector.tensor_scalar_mul(sgn[0:1, :], sgn[0:1, :], -1.0)
        ident = wp.tile([128, 128], bf16, tag="ident")
        nc.scalar.dma_start(out=ident[:, :], in_=ident_in[:, :])

        # ---------------- weights (all host-prepped, straight DMA loads) -----
        # layer-0 weights on the sync queue (first need), rest on scalar
        wld_t, wrd_t, blb_row, brb_row, recipcol = [], [], [], [], []
        for l in range(4):
            din = DIN if l == 0 else H
            nk0 = din // 128
            weng = nc.sync if l == 0 else nc.scalar
            wld = wp.tile([128, nk0, H], bf16, tag=f"wld{l}")
            wrd = wp.tile([128, nk0, H], bf16, tag=f"wrd{l}")
            wld_t.append(wld)
            wrd_t.append(wrd)
            for W_off, wdev in ((OFF[f"wl{l}"], wld), (OFF[f"wr{l}"], wrd)):
                weng.dma_start(out=wdev[:, :, :],
                               in_=wb_in[W_off:W_off + nk0 * 128, :].rearrange(
                                   "(p k) h -> p k h", p=128))
            t = wp.tile([1, H], bf16, tag=f"blb{l}")
            weng.dma_start(out=t[:1, :], in_=wb_in[OFF["blb"] + l:OFF["blb"] + l + 1, :])
            blb_row.append(t)
            t = wp.tile([1, H], bf16, tag=f"brb{l}")
            weng.dma_start(out=t[:1, :], in_=wb_in[OFF["brb"] + l:OFF["brb"] + l + 1, :])
            brb_row.append(t)
        wfcols = wp.tile([128, WF_COLS], f32, tag="wfcols")
        nc.sync.dma_start(out=wfcols[:, :], in_=wf_in[:, :])
        recipcol = [wfcols[:, WF_RECIP + 4 * l:WF_RECIP + 4 * l + 4]
                    for l in range(4)]
        lb1col = wfcols[:, WF_LB1:WF_LB1 + 4]
        lb2col = wfcols[:, WF_LB2:WF_LB2 + 2]
        lb3col = wfcols[:, WF_LB3:WF_LB3 + 1]

        # ---------------- MLP weights ----------------
        lw1_dev = wp.tile([128, 4, H], bf16, tag="lw1")
        nc.scalar.dma_start(out=lw1_dev[:, :, :],
                            in_=wb_in[OFF["lw1"]:OFF["lw1"] + H, :].rearrange(
                                "(p k) h -> p k h", p=128))
        lw2_dev = wp.tile([128, 4, 256], bf16, tag="lw2")
        nc.scalar.dma_start(out=lw2_dev[:, :, :],
                            in_=wb_in[OFF["lw2"]:OFF["lw2"] + H, :256].rearrange(
                                "(p k) h -> p k h", p=128))
        lw3_dev = wp.tile([128, 2, 2], bf16, tag="lw3")
        nc.scalar.dma_start(out=lw3_dev[:, :, :],
                            in_=wb_in[OFF["lw3"]:OFF["lw3"] + 256, :2].rearrange(
                                "(p k) h -> p k h", p=128))

        # edge-phase constants: big loads deferred here so the sync queue
        # serves layer-0 weights/x first (edge_a needs these only after the
        # first node blocks).
        srcidx = wp.tile([128, NT * 8], i16, tag="srcidx")
        nc.scalar.dma_start(out=srcidx[:, :], in_=srcidx_in[:, :])
        e01T_sb = wp.tile([128, NT * 128], bf16, tag="e01T")
        nc.scalar.dma_start(out=e01T_sb[:, :], in_=e01T_in[:, :])
        e01_sb = wp.tile([128, NT * BLK], bf16, tag="e01")
        nc.scalar.dma_start(out=e01_sb[:, :], in_=e01_in[:, :])

        # xr for the current/next layer's own nodes (rotation across layers)
        xr_cur = [np2.tile([128, NBLK, H], bf16, tag="xr", name=f"xr{i}")
                  for i in range(2)]

        # ---------------- helpers ----------------
        def node_block(l, b, lhsT_fn, nk):
            """xl/xr for dst block b of layer l from feature-major lhsT chunks."""
            pxl = ps.tile([128, H], f32, tag="pnl", bufs=2)
            pxr = ps.tile([128, H], f32, tag="pnr")
            for k in range(nk):
                lhsT = lhsT_fn(k)
                nc.tensor.matmul(pxl[:BLK, :], lhsT, wld_t[l][:, k, :],
                                 start=(k == 0), stop=False, skip_group_check=True)
                nc.tensor.matmul(pxr[:BLK, :], lhsT, wrd_t[l][:, k, :],
                                 start=(k == 0), stop=False, skip_group_check=True)
            nc.tensor.matmul(pxl[:BLK, :], onesrow[:1, :BLK], blb_row[l][:1, :],
                             start=False, stop=True, skip_group_check=True)
            nc.tensor.matmul(pxr[:BLK, :], onesrow[:1, :BLK], brb_row[l][:1, :],
                             start=False, stop=True, skip_group_check=True)
            xl_blk = np_.tile([128, H], bf16, tag="xlblk", bufs=2)
            nc.vector.tensor_copy(xl_blk[:BLK, :], pxl[:BLK, :])
            nc.vector.tensor_copy(xr_cur[l % 2][:BLK, b, :], pxr[:BLK, :])
            eng = nc.sync if b % 2 == 0 else nc.scalar
            eng.dma_start(out=xl_loc[l][b * BLK:(b + 1) * BLK, :],
                          in_=xl_blk[:BLK, :])

        def ag_chunk(l, ch):
            if single_core:
                for cc in range(NC):
                    eng = nc.sync if cc % 2 == 0 else nc.scalar
                    eng.dma_start(
                        out=xl_full[l][ch * NC * CROWS + cc * CROWS:
                                       ch * NC * CROWS + (cc + 1) * CROWS, :],
                        in_=xl_loc[l][ch * CROWS:(ch + 1) * CROWS, :])
            else:
                nc.gpsimd.collective_compute(
                    "AllGather", ALU.bypass,
                    replica_groups=[list(range(NC))],
                    ins=[xl_loc[l][ch * CROWS:(ch + 1) * CROWS, :]],
                    outs=[xl_full[l][ch * NC * CROWS:(ch + 1) * NC * CROWS, :]],
                )

        def edge_a(l, b):
            """Stage A for dst block b: split gathers (leading tiles depend
            only on early AG chunks); per tile, accumulate u = xr[dst]
            (one-hot matmul) + xlg (ident matmul) in PSUM, one full-width
            Prelu with fused accum row-sum (S_all) + DVE reduce of the
            neg-att slab; e = S_all - 2*S_neg. Returns (xlg, esum)."""
            TB = TBs[b]
            c0 = cum[b]
            kp = KP[l]
            xlg = gp.tile([128, TBMAX, H], bf16, tag="xlg")
            # split gathers by required AG chunk: fine (5-way) for layer 0
            # where the serial AG chain is exposed, coarse (early/late)
            # after; each dma_gather costs ~1us of Q7 descgen.
            if l == 0:
                splits = [(GB[b][ch], ch) for ch in range(CH)]
            else:
                splits = [(GB[b][CH - 2], CH - 2), (GB[b][CH - 1], CH - 1)]
            t0 = 0
            for t1, ch in splits:
                if t1 <= t0:
                    continue
                rows = (ch + 1) * NC * CROWS
                nc.gpsimd.dma_gather(
                    out_ap=xlg[:, t0:t1, :], in_ap=xl_full[l][0:rows, :],
                    idxs_ap=srcidx[:, (c0 + t0) * 8:(c0 + t1) * 8],
                    num_idxs=(t1 - t0) * 128, num_idxs_reg=(t1 - t0) * 128,
                    elem_size=H, single_packet=False, queue_num=b % 2)
                t0 = t1
            es_all = ep.tile([128, TBMAX], f32, tag="esall", bufs=2)
            es_neg = ep.tile([128, TBMAX], f32, tag="esneg", bufs=2)
            for t in range(TB):
                pbc = ps2.tile([128, H], f32, tag="pbc")
                nc.tensor.matmul(pbc[:, :],
                                 e01T_sb[:BLK, (c0 + t) * 128:(c0 + t + 1) * 128],
                                 xr_cur[l % 2][:BLK, b, :],
                                 start=True, stop=False, skip_group_check=True)
                nc.tensor.matmul(pbc[:, :], ident[:, :], xlg[:, t, :],
                                 start=False, stop=True, skip_group_check=True)
                wscr = ep.tile([128, H], bf16, tag="wscr", bufs=2)
                nc.scalar.activation(wscr[:, :], pbc[:, :], AF.Prelu, alpha=NEG,
                                     accum_out=es_all[:, t:t + 1])
                if kp < H:
                    nc.vector.tensor_reduce(es_neg[:, t:t + 1], wscr[:, kp:H],
                                            axis=mybir.AxisListType.X, op=ALU.add)
            esum = ep.tile([128, TBMAX], f32, tag="eavg", bufs=2)
            if kp == H:
                esum = es_all
            else:
                nc.vector.tensor_scalar_mul(es_neg[:, :TB], es_neg[:, :TB], -2.0)
                nc.vector.tensor_add(esum[:, :TB], es_all[:, :TB], es_neg[:, :TB])
            return xlg, esum

        def edge_b(l, b, ctx, hT_out):
            """Stage B: exp, scatter-matmul aggregation, relu (DVE), transposes
            with 1/beta fold into hT_out."""
            TB = TBs[b]
            c0 = cum[b]
            xlg, esum = ctx
            pbuf = ep.tile([128, TBMAX], f32, tag="pbuf")
            nc.scalar.activation(pbuf[:, :TB], esum[:, :TB], AF.Exp)
            pf = ps.tile([128, H], f32, tag="pf")
            ps1 = ps.tile([128, 1], f32, tag="ps1")
            for t in range(TB):
                S = ep.tile([128, BLK], bf16, tag="S")
                nc.vector.tensor_scalar_mul(
                    S[:, :], e01_sb[:, (c0 + t) * BLK:(c0 + t + 1) * BLK],
                    pbuf[:, t:t + 1])
                nc.tensor.matmul(pf[:BLK, :], S[:, :], xlg[:, t, :],
                                 start=(t == 0), stop=(t == TB - 1),
                                 skip_group_check=True)
                nc.tensor.matmul(ps1[:BLK, :1], S[:, :], ones128[:, :1],
                                 start=(t == 0), stop=(t == TB - 1),
                                 skip_group_check=True)
            srec = ep.tile([128, 1], f32, tag="srec")
            nc.vector.reciprocal(srec[:BLK, :], ps1[:BLK, :1])
            hb = ep.tile([128, H], bf16, tag="hb", bufs=1)
            nc.vector.tensor_scalar(hb[:BLK, :], pf[:BLK, :], srec[:BLK, :], 0.0,
                                    op0=ALU.mult, op1=ALU.max)
            for kc in range(4):
                ptr = ps2.tile([128, 128], bf16, tag="ptr", bufs=1)
                nc.tensor.transpose(ptr[:, :BLK], hb[:BLK, ts(kc, 128)],
                                    ident[:BLK, :BLK])
                nc.vector.tensor_scalar_mul(hT_out[:, kc, b * BLK:(b + 1) * BLK],
                                            ptr[:, :BLK], recipcol[l][:, kc:kc + 1])

        # ---------------- MLP head (per 128-node chunk) ----------------
        jchunks = [(j0, min(128, NLOC - j0)) for j0 in range(0, NLOC, 128)]
        mlp_ready_at = {}
        for _j, (_j0, _w) in enumerate(jchunks):
            mlp_ready_at.setdefault((_j0 + _w - 1) // BLK, []).append(_j)

        def mlp_chunk(jidx, hT):
            j0, w = jchunks[jidx]
            h1c = np_.tile([128, 4, 128], bf16, tag="h1c", bufs=2)
            for m in range(4):
                pm = ps2.tile([128, H], f32, tag="pbc")
                for k in range(4):
                    nc.tensor.matmul(pm[:, :w], lw1_dev[:, k, ts(m, 128)],
                                     hT[:, k, j0:j0 + w], start=(k == 0),
                                     stop=(k == 3), skip_group_check=True)
                nc.scalar.activation(h1c[:, m, :w], pm[:, :w], AF.Relu,
                                     bias=lb1col[:, m:m + 1])
            h2c = np_.tile([128, 2, 128], bf16, tag="h2c", bufs=2)
            for m in range(2):
                pm = ps2.tile([128, H], f32, tag="pbc")
                for k in range(4):
                    nc.tensor.matmul(pm[:, :w], lw2_dev[:, k, ts(m, 128)],
                                     h1c[:, k, :w], start=(k == 0),
                                     stop=(k == 3), skip_group_check=True)
                nc.scalar.activation(h2c[:, m, :w], pm[:, :w], AF.Relu,
                                     bias=lb2col[:, m:m + 1])
            pm3 = ps2.tile([128, H], f32, tag="pbc")
            for k in range(2):
                nc.tensor.matmul(pm3[:2, :w], lw3_dev[:, k, :],
                                 h2c[:, k, :w], start=(k == 0), stop=(k == 1),
                                 skip_group_check=True)
            logc = np_.tile([2, 128], f32, tag="logc", bufs=2)
            nc.scalar.activation(logc[:2, :w], pm3[:2, :w], AF.Identity,
                                 bias=lb3col[:2, :])
            pd = ps2.tile([128, H], f32, tag="pbc")
            nc.tensor.matmul(pd[:1, :w], sgn[:2, :], logc[:2, :w],
                             start=True, stop=True, skip_group_check=True)
            emd = np_.tile([1, 128], f32, tag="emd", bufs=2)
            nc.scalar.activation(emd[:1, :w], pd[:1, :w], AF.Exp, scale=-1.0)
            p1c = np_.tile([1, 128], f32, tag="p1c", bufs=2)
            nc.vector.tensor_scalar_add(p1c[:1, :w], emd[:1, :w], 1.0)
            nc.vector.reciprocal(p1c[:1, :w], p1c[:1, :w])
            nc.vector.tensor_mul(emd[:1, :w], p1c[:1, :w], emd[:1, :w])
            p0c = emd
            nc.scalar.dma_start(out=logitsT_out[:, j0:j0 + w], in_=logc[:2, :w])
            nc.scalar.dma_start(out=probs0_out[:, j0:j0 + w], in_=p0c[:1, :w])
            nc.scalar.dma_start(out=probs1_out[:, j0:j0 + w], in_=p1c[:1, :w])

        # ---------------- main ----------------
        for rep in range(reps):
            # layer 0: every core computes xl for ALL nodes from the
            # replicated input x (no layer-0 AllGather at all) straight into
            # its local xl_full[0]; xr only for its own dst nodes from the
            # small per-core x.
            # own xr blocks from the per-core x (needed first by edge stage A)
            for b in range(NBLK):
                xc = ep.tile([128, 8, BLK], bf16, tag="xc", bufs=2)
                nc.sync.dma_start(
                    out=xc[:, :, :],
                    in_=x_in[:, b * BLK:(b + 1) * BLK].rearrange(
                        "(k p) n -> p k n", p=128))
                pxr = ps.tile([128, H], f32, tag="pnr")
                for k in range(8):
                    nc.tensor.matmul(pxr[:BLK, :], xc[:, k, :BLK], wrd_t[0][:, k, :],
                                     start=(k == 0), stop=False, skip_group_check=True)
                nc.tensor.matmul(pxr[:BLK, :], onesrow[:1, :BLK], brb_row[0][:1, :],
                                 start=False, stop=True, skip_group_check=True)
                nc.vector.tensor_copy(xr_cur[0][:BLK, b, :], pxr[:BLK, :])
            NGB = N // BLK  # 80 global blocks
            for gb in range(NGB):
                xc = ep.tile([128, 8, BLK], bf16, tag="xc", bufs=2)
                nc.sync.dma_start(
                    out=xc[:, :, :],
                    in_=xf_in[:, gb * BLK:(gb + 1) * BLK].rearrange(
                        "(k p) n -> p k n", p=128))
                pxl = ps.tile([128, H], f32, tag="pnl", bufs=2)
                for k in range(8):
                    nc.tensor.matmul(pxl[:BLK, :], xc[:, k, :BLK], wld_t[0][:, k, :],
                                     start=(k == 0), stop=False, skip_group_check=True)
                nc.tensor.matmul(pxl[:BLK, :], onesrow[:1, :BLK], blb_row[0][:1, :],
                                 start=False, stop=True, skip_group_check=True)
                xl_blk = np_.tile([128, H], bf16, tag="xlblk", bufs=2)
                if gb % 2 == 0:
                    nc.vector.tensor_copy(xl_blk[:BLK, :], pxl[:BLK, :])
                else:
                    nc.scalar.activation(xl_blk[:BLK, :], pxl[:BLK, :], AF.Copy)
                eng = nc.sync if gb % 2 == 0 else nc.scalar
                eng.dma_start(out=xl_full[0][gb * BLK:(gb + 1) * BLK, :],
                              in_=xl_blk[:BLK, :])
            hT = None
            for l in range(nlayers):
                if not nedge:
                    continue
                hT_next = np2.tile([128, 4, NLOC], bf16, tag="hT", name=f"hT{rep}_{l}")

                def finish(b, _l=l, _hT=hT_next):
                    edge_b(_l, b, ctxs[b], _hT)
                    if _l + 1 < nlayers:
                        node_block(_l + 1, b,
                                   lambda k, _h=_hT, _b=b:
                                   _h[:, k, _b * BLK:(_b + 1) * BLK], 4)
                        if (b + 1) % CBLK == 0:
                            ag_chunk(_l + 1, b // CBLK)
                    else:
                        for j in mlp_ready_at.get(b, []):
                            mlp_chunk(j, _hT)

                ctxs = {}
                for b in range(NBLK):
                    ctxs[b] = edge_a(l, b)
                    if b > 0:
                        finish(b - 1)
                finish(NBLK - 1)
                hT = hT_next

            # ---------------- MLP head (fallback when no edge layers ran) ----
            if hT is None:
                hT = np2.tile([128, 4, NLOC], bf16, tag="hT", name=f"hT{rep}_x")
                nc.vector.memset(hT[:, :, :], 0.0)
                for j in range(len(jchunks)):
                    mlp_chunk(j, hT)

    nc.compile()
    return nc


_CACHE = {}
_LAST_IN_MAPS = None


def _get_program(TBs, KP):
    key = (tuple(TBs), tuple(KP))
    if key not in _CACHE:
        _CACHE[key] = _build(TBs, KP)
    return _CACHE[key]


def _run(inputs, trace=False):
    inp = {k: np.asarray(v) for k, v in inputs.items()}
    x = inp["x"].astype(np.float32)
    edge_index = inp["edge_index"].astype(np.int64)
    TBs, cores, assign = _prep_edges(edge_index)

    perms, KP = [], []
    for l in range(1, 5):
        att = inp[f"att{l}"].astype(np.float32)
        perm = np.argsort(att <= 0, kind="stable")
        perms.append(perm)
        KP.append(int((att > 0).sum()))
    wbf, wf = _pack_weights(inp, perms)
    ident = np.eye(128, dtype=npbf)

    # xf: x columns in global xl_full row order, replicated to every core
    rowof = np.empty(N, np.int64)
    for c in range(NC):
        p = np.arange(NLOC)
        rowof[assign[c]] = (p // CROWS) * (NC * CROWS) + c * CROWS + (p % CROWS)
    node_at_row = np.empty(N, np.int64)
    node_at_row[rowof] = np.arange(N)
    xf = np.ascontiguousarray(x[node_at_row].T.astype(npbf))

    ncprog = _get_program(TBs, KP)
    in_maps = []
    for c in range(NC):
        xT = np.ascontiguousarray(x[assign[c]].T.astype(npbf))
        m = {"x": xT, "xf": xf, "wbf": wbf, "wf32": wf,
             "srcidx": cores[c]["src16"],
             "e01": cores[c]["e01"], "e01T": cores[c]["e01T"], "ident": ident}
        in_maps.append(m)

    global _LAST_IN_MAPS
    _LAST_IN_MAPS = in_maps
    res = run_bass_kernel_spmd(ncprog, in_maps, list(range(NC)), trace=trace)
    logits = np.empty((N, 2), np.float32)
    probs = np.empty((N, 2), np.float32)
    for c in range(NC):
        r = res.results[c]
        logits[assign[c]] = r["logitsT"].T
        probs[assign[c], 0] = r["probs0"][0]
        probs[assign[c], 1] = r["probs1"][0]
    return (logits, probs), res


def kernel(**inputs):
    out, _ = _run(inputs, trace=False)
    return out



# revision 35
# speedup vs baseline: 1.6396x; 1.0604x over previous
"""GATv2 4-layer + MLP head on 8 Trainium2 NeuronCores (Bass/Tile), v3.

Strategy: partition destination nodes across 8 cores (1250 each, degree-
balanced into 10 blocks of 125). All weights host-prepped (beta = max(|att|,
eps) folded into bf16 wl/wr; biases riding along; recip columns f32).

Layer 0: every core computes xl for ALL nodes from the replicated bf16 input
x straight into its local xl_full[0] (no layer-0 collective); xr only for its
own dst nodes. Layers 1-3: xl rows DMA to DRAM and chunked AllGathers overlap
the previous layer's edge compute.

Edge phase per dst block: split SWDGE gathers of xl[src] rows (leading tiles,
host-sorted by source AG chunk, depend only on early chunks -- range-based
deps let them start before the last collective lands); per 128-edge tile the
PE accumulates u = xr[dst] (one-hot E01^T matmul) + xlg (identity matmul) in
PSUM; ONE full-width Prelu (ACT) with fused accum row-sum gives S_all and a
DVE reduce over the negative-att slab gives S_neg; e = S_all - 2*S_neg (via
att.lrelu(z) = sum_pos prelu_.2(beta z) - sum_neg prelu_.2(beta z));
p = exp(e); aggregation via scatter matmul S = E01 * p into PSUM;
h = relu(num/den); hT via PE transposes with 1/beta fold (DVE). The MLP head
chunks interleave with layer 3's edge blocks. Softmax via exp-based sigmoid
(stays in the exp ACT table set).
"""
import sys

sys.path.insert(0, "/opt/trn_rl_repo")

from contextlib import ExitStack

import numpy as np
import ml_dtypes

import concourse.bass as bass
import concourse.bacc as bacc
import concourse.tile as tile
from concourse import mybir
from concourse.bass_utils import run_bass_kernel_spmd

bf16 = mybir.dt.bfloat16
f32 = mybir.dt.float32
i16 = mybir.dt.int16
AF = mybir.ActivationFunctionType
ALU = mybir.AluOpType
ts = bass.ts
npbf = ml_dtypes.bfloat16

N, E, DIN, H = 10000, 80000, 1024, 512
NEG = 0.2
NC = 8
NLOC = N // NC          # 1250 dst nodes per core
BLK = 125               # dst nodes per block
NBLK = NLOC // BLK      # 10 blocks per core
CH = 5                  # AllGather chunks per layer
CROWS = NLOC // CH      # rows per AG chunk (250)
CBLK = NBLK // CH       # blocks per AG chunk (2)

# wbf row offsets (bf16 [WBF_ROWS, 512]): fully host-prepped device weights.
# wld/wrd rows are in (p k) order with beta = max(|att|,eps) folded in and
# the output bias riding along (blb = (bl+b)*beta, brb = (br-b)*beta).
OFF = {}
_o = 0
for _l in range(4):
    din = DIN if _l == 0 else H
    OFF[f"wl{_l}"] = _o; _o += din
    OFF[f"wr{_l}"] = _o; _o += din
OFF["lw1"] = _o; _o += H
OFF["lw2"] = _o; _o += H
OFF["lw3"] = _o; _o += 256
OFF["blb"] = _o; _o += 4
OFF["brb"] = _o; _o += 4
WBF_ROWS = _o
# wf32 [128, WF_COLS] f32 column blob: recip (1/beta) per layer, lb cols
WF_RECIP = 0            # 4 cols per layer (k p) layout
WF_LB1 = 16             # 4
WF_LB2 = 20             # 2
WF_LB3 = 22             # 1 (partitions 0:2)
WF_COLS = 23


# ---------------------------------------------------------------- host prep
def _prep_edges(edge_index):
    src = np.concatenate([edge_index[0], np.arange(N)]).astype(np.int64)
    dst = np.concatenate([edge_index[1], np.arange(N)]).astype(np.int64)
    deg = np.bincount(dst, minlength=N)
    NBUCK = NC * NBLK
    order = np.argsort(-deg, kind="stable")
    bucket_edges = np.zeros(NBUCK, np.int64)
    bucket_nodes = [[] for _ in range(NBUCK)]
    import heapq
    heap = [(0, kk) for kk in range(NBUCK)]
    heapq.heapify(heap)
    for g in order:
        while True:
            w, kk = heapq.heappop(heap)
            if len(bucket_nodes[kk]) < BLK:
                break
        bucket_nodes[kk].append(int(g))
        bucket_edges[kk] = w + int(deg[g])
        if len(bucket_nodes[kk]) < BLK:
            heapq.heappush(heap, (int(bucket_edges[kk]), kk))
    assign = [[] for _ in range(NC)]
    for c in range(NC):
        for b in range(NBLK):
            assign[c].extend(bucket_nodes[c * NBLK + b])
    assign = [np.array(a, np.int64) for a in assign]
    # chunked AG row layout: node at (core c, pos p) lands at
    # (p//CROWS)*(NC*CROWS) + c*CROWS + p%CROWS in xl_full [N, H]
    rowof = np.empty(N, np.int64)
    posof = np.empty(N, np.int64)
    coreof = np.empty(N, np.int64)
    for c in range(NC):
        p = np.arange(NLOC)
        rowof[assign[c]] = (p // CROWS) * (NC * CROWS) + c * CROWS + (p % CROWS)
        posof[assign[c]] = p
        coreof[assign[c]] = c
    percore = []
    for c in range(NC):
        sel = coreof[dst] == c
        s_, d_ = rowof[src[sel]], posof[dst[sel]]
        # within each dst block, order edges by required AG chunk of the src
        # row so leading tiles depend only on early chunks (split gathers)
        o = np.lexsort((s_ // (NC * CROWS), d_ // BLK))
        s_, d_ = s_[o], d_[o]
        blocks = []
        for bb in range(NBLK):
            m = (d_ // BLK) == bb
            blocks.append((s_[m], d_[m] - bb * BLK))
        percore.append(blocks)
    TBs = tuple(max(max(-(-len(percore[c][b][0]) // 128), 1) for c in range(NC))
                for b in range(NBLK))
    cum = np.concatenate([[0], np.cumsum(TBs)]).astype(int)
    NT = int(cum[-1])
    EPAD = NT * 128
    # per block: shared (min-over-cores) tile counts whose srcs all sit in
    # AG chunks <= ch; GB[b] is a nondecreasing list of 5 tile boundaries
    GB = []
    for b in range(NBLK):
        gb = []
        for ch in range(CH):
            lim = (ch + 1) * NC * CROWS
            cnt = min(int((percore[c][b][0] < lim).sum()) // 128
                      for c in range(NC))
            gb.append(cnt)
        gb[-1] = TBs[b]
        GB.append(tuple(gb))
    cores = []
    for c in range(NC):
        src16 = np.zeros(EPAD, np.int16)
        e01 = np.zeros((128, NT * BLK), npbf)
        e01T = np.zeros((128, NT * 128), npbf)
        for b in range(NBLK):
            s, d = percore[c][b]
            n = len(s)
            base = int(cum[b]) * 128
            src16[base:base + n] = s
            tt = int(cum[b]) + np.arange(n) // 128
            pp = np.arange(n) % 128
            e01[pp, tt * BLK + d] = 1.0
            e01T[d, tt * 128 + pp] = 1.0
        w = src16.reshape(-1, 16).T.copy()
        cores.append(dict(src16=np.tile(w, (8, 1)).copy(),
                          e01=np.ascontiguousarray(e01),
                          e01T=np.ascontiguousarray(e01T)))
    return (TBs, tuple(GB)), cores, assign


def _pack_weights(inp, perms):
    """Host-side full weight prep: beta-scaled bf16 blob + small f32 cols."""
    def pk(W):
        nk = W.shape[0] // 128
        return W.reshape(nk, 128, W.shape[1]).transpose(1, 0, 2).reshape(
            nk * 128, W.shape[1])

    wbf = np.zeros((WBF_ROWS, H), npbf)
    wf = np.zeros((128, WF_COLS), np.float32)
    for l in range(4):
        rowp = perms[l - 1] if l > 0 else None
        att = np.asarray(inp[f"att{l + 1}"], np.float32)[perms[l]]
        beta = np.maximum(np.abs(att), 1e-30)
        bb = np.asarray(inp[f"b{l + 1}"], np.float32)[perms[l]]
        bl = np.asarray(inp[f"bl{l + 1}"], np.float32)[perms[l]]
        br = np.asarray(inp[f"br{l + 1}"], np.float32)[perms[l]]
        for nm, bias in (("wl", bl + bb), ("wr", br - bb)):
            W = np.asarray(inp[f"{nm}{l + 1}"], np.float32)
            if rowp is not None:
                W = W[rowp, :]
            W = W[:, perms[l]] * beta[None, :]
            wbf[OFF[f"{nm}{l}"]:OFF[f"{nm}{l}"] + W.shape[0], :] = pk(W)
            off = OFF["blb" if nm == "wl" else "brb"] + l
            wbf[off, :] = bias * beta
        wf[:, WF_RECIP + 4 * l:WF_RECIP + 4 * l + 4] = (
            1.0 / beta).reshape(4, 128).T
    wbf[OFF["lw1"]:OFF["lw1"] + H, :] = pk(
        np.asarray(inp["lw1"], np.float32)[perms[3], :])
    wbf[OFF["lw2"]:OFF["lw2"] + H, :256] = pk(np.asarray(inp["lw2"], np.float32))
    wbf[OFF["lw3"]:OFF["lw3"] + 256, :2] = pk(np.asarray(inp["lw3"], np.float32))
    wf[:, WF_LB1:WF_LB1 + 4] = np.asarray(inp["lb1"], np.float32).reshape(4, 128).T
    wf[:, WF_LB2:WF_LB2 + 2] = np.asarray(inp["lb2"], np.float32).reshape(2, 128).T
    wf[0:2, WF_LB3] = np.asarray(inp["lb3"], np.float32)
    return wbf, wf


# -------------------------------------------------------------- bass program
def _build(TB_info, KP, single_core=False, nlayers=4, nedge=True, reps=1,
           l0ag=False):
    TBs, GB = TB_info
    TBs = tuple(TBs)
    TBMAX = max(TBs)
    cum = [0]
    for t in TBs:
        cum.append(cum[-1] + t)
    NT = cum[-1]
    nc = bacc.Bacc("TRN2", num_swdge_queues=2)
    P = nc.declare_dram_parameter
    xf_in = P("xf", [DIN, N], bf16, isOutput=False)
    x_in = P("x", [DIN, NLOC], bf16, isOutput=False)
    wb_in = P("wbf", [WBF_ROWS, H], bf16, isOutput=False)
    wf_in = P("wf32", [128, WF_COLS], f32, isOutput=False)
    srcidx_in = P("srcidx", [128, NT * 8], i16, isOutput=False)
    e01_in = P("e01", [128, NT * BLK], bf16, isOutput=False)
    e01T_in = P("e01T", [128, NT * 128], bf16, isOutput=False)
    ident_in = P("ident", [128, 128], bf16, isOutput=False)
    logitsT_out = P("logitsT", [2, NLOC], f32, isOutput=True)
    probs0_out = P("probs0", [1, NLOC], f32, isOutput=True)
    probs1_out = P("probs1", [1, NLOC], f32, isOutput=True)

    xl_loc = [nc.dram_tensor(f"xlloc{l}", [NLOC, H], bf16) for l in range(4)]
    xl_full = [nc.dram_tensor(f"xlfull{l}", [N, H], bf16, addr_space="Shared")
               for l in range(4)]

    with tile.TileContext(nc) as tc, ExitStack() as ctx:
        wp = ctx.enter_context(tc.tile_pool(name="wp", bufs=1))
        np_ = ctx.enter_context(tc.tile_pool(name="np", bufs=2))
        np2 = ctx.enter_context(tc.tile_pool(name="np2", bufs=2))
        ep = ctx.enter_context(tc.tile_pool(name="ep", bufs=2))
        gp = ctx.enter_context(tc.tile_pool(name="gp", bufs=2))
        ps = ctx.enter_context(tc.tile_pool(name="ps", bufs=1, space="PSUM"))
        ps2 = ctx.enter_context(tc.tile_pool(name="ps2", bufs=2, space="PSUM"))

        # ---------------- constants ----------------
        ones128 = wp.tile([128, 1], bf16, tag="ones128")
        nc.vector.memset(ones128[:, :], 1.0)
        onesrow = wp.tile([1, 128], bf16, tag="onesrow")
        nc.vector.memset(onesrow[:1, :], 1.0)
        sgn = wp.tile([2, 1], f32, tag="sgn")
        nc.vector.memset(sgn[:2, :], 1.0)
        nc.vector.tensor_scalar_mul(sgn[0:1, :], sgn[0:1, :], -1.0)
        ident = wp.tile([128, 128], bf16, tag="ident")
        nc.scalar.dma_start(out=ident[:, :], in_=ident_in[:, :])

        # ---------------- weights (all host-prepped, straight DMA loads) -----
        # layer-0 weights on the sync queue (first need), rest on scalar
        wld_t, wrd_t, blb_row, brb_row, recipcol = [], [], [], [], []
        for l in range(4):
            din = DIN if l == 0 else H
            nk0 = din // 128
            weng = nc.sync if l == 0 else nc.scalar
            wld = wp.tile([128, nk0, H], bf16, tag=f"wld{l}")
            wrd = wp.tile([128, nk0, H], bf16, tag=f"wrd{l}")
            wld_t.append(wld)
            wrd_t.append(wrd)
            for W_off, wdev in ((OFF[f"wl{l}"], wld), (OFF[f"wr{l}"], wrd)):
                weng.dma_start(out=wdev[:, :, :],
                               in_=wb_in[W_off:W_off + nk0 * 128, :].rearrange(
                                   "(p k) h -> p k h", p=128))
            t = wp.tile([1, H], bf16, tag=f"blb{l}")
            weng.dma_start(out=t[:1, :], in_=wb_in[OFF["blb"] + l:OFF["blb"] + l + 1, :])
            blb_row.append(t)
            t = wp.tile([1, H], bf16, tag=f"brb{l}")
            weng.dma_start(out=t[:1, :], in_=wb_in[OFF["brb"] + l:OFF["brb"] + l + 1, :])
            brb_row.append(t)
        wfcols = wp.tile([128, WF_COLS], f32, tag="wfcols")
        nc.sync.dma_start(out=wfcols[:, :], in_=wf_in[:, :])
        recipcol = [wfcols[:, WF_RECIP + 4 * l:WF_RECIP + 4 * l + 4]
                    for l in range(4)]
        lb1col = wfcols[:, WF_LB1:WF_LB1 + 4]
        lb2col = wfcols[:, WF_LB2:WF_LB2 + 2]
        lb3col = wfcols[:, WF_LB3:WF_LB3 + 1]

        # ---------------- MLP weights ----------------
        lw1_dev = wp.tile([128, 4, H], bf16, tag="lw1")
        nc.scalar.dma_start(out=lw1_dev[:, :, :],
                            in_=wb_in[OFF["lw1"]:OFF["lw1"] + H, :].rearrange(
                                "(p k) h -> p k h", p=128))
        lw2_dev = wp.tile([128, 4, 256], bf16, tag="lw2")
        nc.scalar.dma_start(out=lw2_dev[:, :, :],
                            in_=wb_in[OFF["lw2"]:OFF["lw2"] + H, :256].rearrange(
                                "(p k) h -> p k h", p=128))
        lw3_dev = wp.tile([128, 2, 2], bf16, tag="lw3")
        nc.scalar.dma_start(out=lw3_dev[:, :, :],
                            in_=wb_in[OFF["lw3"]:OFF["lw3"] + 256, :2].rearrange(
                                "(p k) h -> p k h", p=128))

        # edge-phase constants: big loads deferred here so the sync queue
        # serves layer-0 weights/x first (edge_a needs these only after the
        # first node blocks).
        srcidx = wp.tile([128, NT * 8], i16, tag="srcidx")
        nc.scalar.dma_start(out=srcidx[:, :], in_=srcidx_in[:, :])
        e01T_sb = wp.tile([128, NT * 128], bf16, tag="e01T")
        nc.scalar.dma_start(out=e01T_sb[:, :], in_=e01T_in[:, :])
        e01_sb = wp.tile([128, NT * BLK], bf16, tag="e01")
        nc.scalar.dma_start(out=e01_sb[:, :], in_=e01_in[:, :])

        # xr for the current/next layer's own nodes (rotation across layers)
        xr_cur = [np2.tile([128, NBLK, H], bf16, tag="xr", name=f"xr{i}")
                  for i in range(2)]

        # ---------------- helpers ----------------
        def node_block(l, b, lhsT_fn, nk):
            """xl/xr for dst block b of layer l from feature-major lhsT chunks."""
            pxl = ps.tile([128, H], f32, tag="pnl", bufs=2)
            pxr = ps.tile([128, H], f32, tag="pnr")
            for k in range(nk):
                lhsT = lhsT_fn(k)
                nc.tensor.matmul(pxl[:BLK, :], lhsT, wld_t[l][:, k, :],
                                 start=(k == 0), stop=False, skip_group_check=True)
                nc.tensor.matmul(pxr[:BLK, :], lhsT, wrd_t[l][:, k, :],
                                 start=(k == 0), stop=False, skip_group_check=True)
            nc.tensor.matmul(pxl[:BLK, :], onesrow[:1, :BLK], blb_row[l][:1, :],
                             start=False, stop=True, skip_group_check=True)
            nc.tensor.matmul(pxr[:BLK, :], onesrow[:1, :BLK], brb_row[l][:1, :],
                             start=False, stop=True, skip_group_check=True)
            xl_blk = np_.tile([128, H], bf16, tag="xlblk", bufs=2)
            nc.vector.tensor_copy(xl_blk[:BLK, :], pxl[:BLK, :])
            nc.vector.tensor_copy(xr_cur[l % 2][:BLK, b, :], pxr[:BLK, :])
            eng = nc.sync if b % 2 == 0 else nc.scalar
            eng.dma_start(out=xl_loc[l][b * BLK:(b + 1) * BLK, :],
                          in_=xl_blk[:BLK, :])

        def ag_chunk(l, ch):
            if single_core:
                for cc in range(NC):
                    eng = nc.sync if cc % 2 == 0 else nc.scalar
                    eng.dma_start(
                        out=xl_full[l][ch * NC * CROWS + cc * CROWS:
                                       ch * NC * CROWS + (cc + 1) * CROWS, :],
                        in_=xl_loc[l][ch * CROWS:(ch + 1) * CROWS, :])
            else:
                nc.gpsimd.collective_compute(
                    "AllGather", ALU.bypass,
                    replica_groups=[list(range(NC))],
                    ins=[xl_loc[l][ch * CROWS:(ch + 1) * CROWS, :]],
                    outs=[xl_full[l][ch * NC * CROWS:(ch + 1) * NC * CROWS, :]],
                )

        def edge_a(l, b):
            """Stage A for dst block b: split gathers (leading tiles depend
            only on early AG chunks); per tile, accumulate u = xr[dst]
            (one-hot matmul) + xlg (ident matmul) in PSUM, one full-width
            Prelu with fused accum row-sum (S_all) + DVE reduce of the
            neg-att slab; e = S_all - 2*S_neg. Returns (xlg, esum)."""
            TB = TBs[b]
            c0 = cum[b]
            kp = KP[l]
            xlg = gp.tile([128, TBMAX, H], bf16, tag="xlg")
            # split gathers by required AG chunk: fine (5-way) for layer 0
            # where the serial AG chain is exposed, coarse (early/late)
            # after; each dma_gather costs ~1us of Q7 descgen.
            if l == 0:
                splits = [(GB[b][ch], ch) for ch in range(CH)]
            else:
                splits = [(GB[b][CH - 2], CH - 2), (GB[b][CH - 1], CH - 1)]
            t0 = 0
            for t1, ch in splits:
                if t1 <= t0:
                    continue
                rows = (ch + 1) * NC * CROWS
                nc.gpsimd.dma_gather(
                    out_ap=xlg[:, t0:t1, :], in_ap=xl_full[l][0:rows, :],
                    idxs_ap=srcidx[:, (c0 + t0) * 8:(c0 + t1) * 8],
                    num_idxs=(t1 - t0) * 128, num_idxs_reg=(t1 - t0) * 128,
                    elem_size=H, single_packet=False, queue_num=b % 2)
                t0 = t1
            es_all = ep.tile([128, TBMAX], f32, tag="esall", bufs=2)
            es_neg = ep.tile([128, TBMAX], f32, tag="esneg", bufs=2)
            for t in range(TB):
                pbc = ps2.tile([128, H], f32, tag="pbc")
                nc.tensor.matmul(pbc[:, :],
                                 e01T_sb[:BLK, (c0 + t) * 128:(c0 + t + 1) * 128],
                                 xr_cur[l % 2][:BLK, b, :],
                                 start=True, stop=False, skip_group_check=True)
                nc.tensor.matmul(pbc[:, :], ident[:, :], xlg[:, t, :],
                                 start=False, stop=True, skip_group_check=True)
                wscr = ep.tile([128, H], bf16, tag="wscr", bufs=2)
                nc.scalar.activation(wscr[:, :], pbc[:, :], AF.Prelu, alpha=NEG,
                                     accum_out=es_all[:, t:t + 1])
                if kp < H:
                    nc.vector.tensor_reduce(es_neg[:, t:t + 1], wscr[:, kp:H],
                                            axis=mybir.AxisListType.X, op=ALU.add)
            esum = ep.tile([128, TBMAX], f32, tag="eavg", bufs=2)
            if kp == H:
                esum = es_all
            else:
                nc.vector.tensor_scalar_mul(es_neg[:, :TB], es_neg[:, :TB], -2.0)
                nc.vector.tensor_add(esum[:, :TB], es_all[:, :TB], es_neg[:, :TB])
            return xlg, esum

        def edge_b(l, b, ctx, hT_out):
            """Stage B: exp, scatter-matmul aggregation, relu (DVE), transposes
            with 1/beta fold into hT_out."""
            TB = TBs[b]
            c0 = cum[b]
            xlg, esum = ctx
            pbuf = ep.tile([128, TBMAX], f32, tag="pbuf")
            nc.scalar.activation(pbuf[:, :TB], esum[:, :TB], AF.Exp)
            pf = ps.tile([128, H], f32, tag="pf")
            ps1 = ps.tile([128, 1], f32, tag="ps1")
            for t in range(TB):
                S = ep.tile([128, BLK], bf16, tag="S")
                nc.vector.tensor_scalar_mul(
                    S[:, :], e01_sb[:, (c0 + t) * BLK:(c0 + t + 1) * BLK],
                    pbuf[:, t:t + 1])
                nc.tensor.matmul(pf[:BLK, :], S[:, :], xlg[:, t, :],
                                 start=(t == 0), stop=(t == TB - 1),
                                 skip_group_check=True)
                nc.tensor.matmul(ps1[:BLK, :1], S[:, :], ones128[:, :1],
                                 start=(t == 0), stop=(t == TB - 1),
                                 skip_group_check=True)
            srec = ep.tile([128, 1], f32, tag="srec")
            nc.vector.reciprocal(srec[:BLK, :], ps1[:BLK, :1])
            hb = ep.tile([128, H], bf16, tag="hb", bufs=1)
            nc.vector.tensor_scalar(hb[:BLK, :], pf[:BLK, :], srec[:BLK, :], 0.0,
                                    op0=ALU.mult, op1=ALU.max)
            for kc in range(4):
                ptr = ps2.tile([128, 128], bf16, tag="ptr", bufs=1)
                nc.tensor.transpose(ptr[:, :BLK], hb[:BLK, ts(kc, 128)],
                                    ident[:BLK, :BLK])
                nc.vector.tensor_scalar_mul(hT_out[:, kc, b * BLK:(b + 1) * BLK],
                                            ptr[:, :BLK], recipcol[l][:, kc:kc + 1])

        # ---------------- MLP head (per 128-node chunk) ----------------
        jchunks = [(j0, min(128, NLOC - j0)) for j0 in range(0, NLOC, 128)]
        mlp_ready_at = {}
        for _j, (_j0, _w) in enumerate(jchunks):
            mlp_ready_at.setdefault((_j0 + _w - 1) // BLK, []).append(_j)

        def mlp_chunk(jidx, hT):
            j0, w = jchunks[jidx]
            h1c = np_.tile([128, 4, 128], bf16, tag="h1c", bufs=2)
            for m in range(4):
                pm = ps2.tile([128, H], f32, tag="pbc")
                for k in range(4):
                    nc.tensor.matmul(pm[:, :w], lw1_dev[:, k, ts(m, 128)],
                                     hT[:, k, j0:j0 + w], start=(k == 0),
                                     stop=(k == 3), skip_group_check=True)
                nc.scalar.activation(h1c[:, m, :w], pm[:, :w], AF.Relu,
                                     bias=lb1col[:, m:m + 1])
            h2c = np_.tile([128, 2, 128], bf16, tag="h2c", bufs=2)
            for m in range(2):
                pm = ps2.tile([128, H], f32, tag="pbc")
                for k in range(4):
                    nc.tensor.matmul(pm[:, :w], lw2_dev[:, k, ts(m, 128)],
                                     h1c[:, k, :w], start=(k == 0),
                                     stop=(k == 3), skip_group_check=True)
                nc.scalar.activation(h2c[:, m, :w], pm[:, :w], AF.Relu,
                                     bias=lb2col[:, m:m + 1])
            pm3 = ps2.tile([128, H], f32, tag="pbc")
            for k in range(2):
                nc.tensor.matmul(pm3[:2, :w], lw3_dev[:, k, :],
                                 h2c[:, k, :w], start=(k == 0), stop=(k == 1),
                                 skip_group_check=True)
            logc = np_.tile([2, 128], f32, tag="logc", bufs=2)
            nc.scalar.activation(logc[:2, :w], pm3[:2, :w], AF.Identity,
                                 bias=lb3col[:2, :])
            pd = ps2.tile([128, H], f32, tag="pbc")
            nc.tensor.matmul(pd[:1, :w], sgn[:2, :], logc[:2, :w],
                             start=True, stop=True, skip_group_check=True)
            emd = np_.tile([1, 128], f32, tag="emd", bufs=2)
            nc.scalar.activation(emd[:1, :w], pd[:1, :w], AF.Exp, scale=-1.0)
            p1c = np_.tile([1, 128], f32, tag="p1c", bufs=2)
            nc.vector.tensor_scalar_add(p1c[:1, :w], emd[:1, :w], 1.0)
            nc.vector.reciprocal(p1c[:1, :w], p1c[:1, :w])
            nc.vector.tensor_mul(emd[:1, :w], p1c[:1, :w], emd[:1, :w])
            p0c = emd
            nc.scalar.dma_start(out=logitsT_out[:, j0:j0 + w], in_=logc[:2, :w])
            nc.scalar.dma_start(out=probs0_out[:, j0:j0 + w], in_=p0c[:1, :w])
            nc.scalar.dma_start(out=probs1_out[:, j0:j0 + w], in_=p1c[:1, :w])

        # ---------------- main ----------------
        for rep in range(reps):
            # layer 0: every core computes xl for ALL nodes from the
            # replicated input x (no layer-0 AllGather at all) straight into
            # its local xl_full[0]; xr only for its own dst nodes from the
            # small per-core x.
            if l0ag:
                # layer 0 node phase for own nodes only + AllGather chunks
                for b in range(NBLK):
                    xc = ep.tile([128, 8, BLK], bf16, tag="xc", bufs=2)
                    nc.sync.dma_start(
                        out=xc[:, :, :],
                        in_=x_in[:, b * BLK:(b + 1) * BLK].rearrange(
                            "(k p) n -> p k n", p=128))
                    node_block(0, b, lambda k, _xc=xc: _xc[:, k, :BLK], 8)
                    if (b + 1) % CBLK == 0:
                        ag_chunk(0, b // CBLK)
            else:
                # own xr blocks from the per-core x (needed first by stage A)
                for b in range(NBLK):
                    xc = ep.tile([128, 8, BLK], bf16, tag="xc", bufs=2)
                    nc.sync.dma_start(
                        out=xc[:, :, :],
                        in_=x_in[:, b * BLK:(b + 1) * BLK].rearrange(
                            "(k p) n -> p k n", p=128))
                    pxr = ps.tile([128, H], f32, tag="pnr")
                    for k in range(8):
                        nc.tensor.matmul(pxr[:BLK, :], xc[:, k, :BLK],
                                         wrd_t[0][:, k, :], start=(k == 0),
                                         stop=False, skip_group_check=True)
                    nc.tensor.matmul(pxr[:BLK, :], onesrow[:1, :BLK],
                                     brb_row[0][:1, :], start=False, stop=True,
                                     skip_group_check=True)
                    nc.vector.tensor_copy(xr_cur[0][:BLK, b, :], pxr[:BLK, :])
                # xl for ALL nodes from the replicated xf (no layer-0 AG)
                NGB = N // BLK  # 80 global blocks
                for gb in range(NGB):
                    xc = ep.tile([128, 8, BLK], bf16, tag="xc", bufs=2)
                    nc.sync.dma_start(
                        out=xc[:, :, :],
                        in_=xf_in[:, gb * BLK:(gb + 1) * BLK].rearrange(
                            "(k p) n -> p k n", p=128))
                    pxl = ps.tile([128, H], f32, tag="pnl", bufs=2)
                    for k in range(8):
                        nc.tensor.matmul(pxl[:BLK, :], xc[:, k, :BLK],
                                         wld_t[0][:, k, :], start=(k == 0),
                                         stop=False, skip_group_check=True)
                    nc.tensor.matmul(pxl[:BLK, :], onesrow[:1, :BLK],
                                     blb_row[0][:1, :], start=False, stop=True,
                                     skip_group_check=True)
                    xl_blk = np_.tile([128, H], bf16, tag="xlblk", bufs=2)
                    if gb % 2 == 0:
                        nc.vector.tensor_copy(xl_blk[:BLK, :], pxl[:BLK, :])
                    else:
                        nc.scalar.activation(xl_blk[:BLK, :], pxl[:BLK, :], AF.Copy)
                    eng = nc.sync if gb % 2 == 0 else nc.scalar
                    eng.dma_start(out=xl_full[0][gb * BLK:(gb + 1) * BLK, :],
                                  in_=xl_blk[:BLK, :])
            hT = None
            for l in range(nlayers):
                if not nedge:
                    continue
                hT_next = np2.tile([128, 4, NLOC], bf16, tag="hT", name=f"hT{rep}_{l}")

                def finish(b, _l=l, _hT=hT_next):
                    edge_b(_l, b, ctxs[b], _hT)
                    if _l + 1 < nlayers:
                        node_block(_l + 1, b,
                                   lambda k, _h=_hT, _b=b:
                                   _h[:, k, _b * BLK:(_b + 1) * BLK], 4)
                        if (b + 1) % CBLK == 0:
                            ag_chunk(_l + 1, b // CBLK)
                    else:
                        for j in mlp_ready_at.get(b, []):
                            mlp_chunk(j, _hT)

                ctxs = {}
                for b in range(NBLK):
                    ctxs[b] = edge_a(l, b)
                    if b > 0:
                        finish(b - 1)
                finish(NBLK - 1)
                hT = hT_next

            # ---------------- MLP head (fallback when no edge layers ran) ----
            if hT is None:
                hT = np2.tile([128, 4, NLOC], bf16, tag="hT", name=f"hT{rep}_x")
                nc.vector.memset(hT[:, :, :], 0.0)
                for j in range(len(jchunks)):
                    mlp_chunk(j, hT)

    nc.compile()
    return nc


_CACHE = {}
_LAST_IN_MAPS = None


def _get_program(TBs, KP):
    key = (tuple(TBs), tuple(KP))
    if key not in _CACHE:
        _CACHE[key] = _build(TBs, KP)
    return _CACHE[key]


def _run(inputs, trace=False):
    inp = {k: np.asarray(v) for k, v in inputs.items()}
    x = inp["x"].astype(np.float32)
    edge_index = inp["edge_index"].astype(np.int64)
    TBs, cores, assign = _prep_edges(edge_index)

    perms, KP = [], []
    for l in range(1, 5):
        att = inp[f"att{l}"].astype(np.float32)
        perm = np.argsort(att <= 0, kind="stable")
        perms.append(perm)
        KP.append(int((att > 0).sum()))
    wbf, wf = _pack_weights(inp, perms)
    ident = np.eye(128, dtype=npbf)

    # xf: x columns in global xl_full row order, replicated to every core
    rowof = np.empty(N, np.int64)
    for c in range(NC):
        p = np.arange(NLOC)
        rowof[assign[c]] = (p // CROWS) * (NC * CROWS) + c * CROWS + (p % CROWS)
    node_at_row = np.empty(N, np.int64)
    node_at_row[rowof] = np.arange(N)
    xf = np.ascontiguousarray(x[node_at_row].T.astype(npbf))

    ncprog = _get_program(TBs, KP)
    in_maps = []
    for c in range(NC):
        xT = np.ascontiguousarray(x[assign[c]].T.astype(npbf))
        m = {"x": xT, "xf": xf, "wbf": wbf, "wf32": wf,
             "srcidx": cores[c]["src16"],
             "e01": cores[c]["e01"], "e01T": cores[c]["e01T"], "ident": ident}
        in_maps.append(m)

    global _LAST_IN_MAPS
    _LAST_IN_MAPS = in_maps
    res = run_bass_kernel_spmd(ncprog, in_maps, list(range(NC)), trace=trace)
    logits = np.empty((N, 2), np.float32)
    probs = np.empty((N, 2), np.float32)
    for c in range(NC):
        r = res.results[c]
        logits[assign[c]] = r["logitsT"].T
        probs[assign[c], 0] = r["probs0"][0]
        probs[assign[c], 1] = r["probs1"][0]
    return (logits, probs), res


def kernel(**inputs):
    out, _ = _run(inputs, trace=False)
    return out



# revision 41
# speedup vs baseline: 2.9634x; 1.8073x over previous
"""GATv2 4-layer + MLP head on 8 Trainium2 NeuronCores (Bass/Tile), v3.

Strategy: partition destination nodes across 8 cores (1250 each, degree-
balanced into 10 blocks of 125). All weights host-prepped (beta = max(|att|,
eps) folded into bf16 wl/wr; biases riding along; recip columns f32).

Layer 0: every core computes xl for ALL nodes from the replicated bf16 input
x straight into its local xl_full[0] (no layer-0 collective); xr only for its
own dst nodes. Layers 1-3: xl rows DMA to DRAM and chunked AllGathers overlap
the previous layer's edge compute.

Edge phase per dst block: split SWDGE gathers of xl[src] rows (leading tiles,
host-sorted by source AG chunk, depend only on early chunks -- range-based
deps let them start before the last collective lands); per 128-edge tile the
PE accumulates u = xr[dst] (one-hot E01^T matmul) + xlg (identity matmul) in
PSUM; ONE full-width Prelu (ACT) with fused accum row-sum gives S_all and a
DVE reduce over the negative-att slab gives S_neg; e = S_all - 2*S_neg (via
att.lrelu(z) = sum_pos prelu_.2(beta z) - sum_neg prelu_.2(beta z));
p = exp(e); aggregation via scatter matmul S = E01 * p into PSUM;
h = relu(num/den); hT via PE transposes with 1/beta fold (DVE). The MLP head
chunks interleave with layer 3's edge blocks. Softmax via exp-based sigmoid
(stays in the exp ACT table set).
"""
import sys

sys.path.insert(0, "/opt/trn_rl_repo")

from contextlib import ExitStack

import numpy as np
import ml_dtypes

import concourse.bass as bass
import concourse.bacc as bacc
import concourse.tile as tile
from concourse import mybir
from concourse.bass_utils import run_bass_kernel_spmd

bf16 = mybir.dt.bfloat16
f32 = mybir.dt.float32
i16 = mybir.dt.int16
AF = mybir.ActivationFunctionType
ALU = mybir.AluOpType
ts = bass.ts
npbf = ml_dtypes.bfloat16
npf8 = ml_dtypes.float8_e4m3
f8 = mybir.dt.float8e4

N, E, DIN, H = 10000, 80000, 1024, 512
NEG = 0.2
NC = 8
NLOC = N // NC          # 1250 dst nodes per core
BLK = 125               # dst nodes per block
NBLK = NLOC // BLK      # 10 blocks per core
CH = 5                  # AllGather chunks per layer
CROWS = NLOC // CH      # rows per AG chunk (250)
CBLK = NBLK // CH       # blocks per AG chunk (2)

# wbf row offsets (bf16 [WBF_ROWS, 512]): fully host-prepped device weights.
# wld/wrd rows are in (p k) order with beta = max(|att|,eps) folded in and
# the output bias riding along (blb = (bl+b)*beta, brb = (br-b)*beta).
OFF = {}
_o = 0
for _l in range(4):
    din = DIN if _l == 0 else H
    OFF[f"wl{_l}"] = _o; _o += din
    OFF[f"wr{_l}"] = _o; _o += din
OFF["lw1"] = _o; _o += H
OFF["lw2"] = _o; _o += H
OFF["lw3"] = _o; _o += 256
OFF["blb"] = _o; _o += 4
OFF["brb"] = _o; _o += 4
WBF_ROWS = _o
# wf32 [128, WF_COLS] f32 column blob: recip (1/beta) per layer, lb cols
WF_RECIP = 0            # 4 cols per layer (k p) layout
WF_LB1 = 16             # 4
WF_LB2 = 20             # 2
WF_LB3 = 22             # 1 (partitions 0:2)
WF_COLS = 23


# ---------------------------------------------------------------- host prep
def _prep_edges(edge_index):
    src = np.concatenate([edge_index[0], np.arange(N)]).astype(np.int64)
    dst = np.concatenate([edge_index[1], np.arange(N)]).astype(np.int64)
    deg = np.bincount(dst, minlength=N)
    NBUCK = NC * NBLK
    order = np.argsort(-deg, kind="stable")
    # tiered packing: the 2000 lowest-OUT-degree nodes go to the "late"
    # buckets (block indices 8,9 = the last AG chunk), so few edge tiles
    # depend on the final, latest-landing collective chunk. In-degree is
    # balanced greedily within each tier (in/out degree are independent).
    outdeg = np.bincount(src, minlength=N)
    late_set = np.zeros(N, bool)
    late_set[np.argsort(outdeg, kind="stable")[:NC * 2 * BLK]] = True
    bucket_edges = np.zeros(NBUCK, np.int64)
    bucket_nodes = [[] for _ in range(NBUCK)]
    import heapq
    late_ids = {c * NBLK + b for c in range(NC) for b in (NBLK - 2, NBLK - 1)}
    heaps = {False: [(0, kk) for kk in range(NBUCK) if kk not in late_ids],
             True: [(0, kk) for kk in sorted(late_ids)]}
    for h in heaps.values():
        heapq.heapify(h)
    for g in order:
        heap = heaps[bool(late_set[g])]
        while True:
            w, kk = heapq.heappop(heap)
            if len(bucket_nodes[kk]) < BLK:
                break
        bucket_nodes[kk].append(int(g))
        bucket_edges[kk] = w + int(deg[g])
        if len(bucket_nodes[kk]) < BLK:
            heapq.heappush(heap, (int(bucket_edges[kk]), kk))
    assign = [[] for _ in range(NC)]
    for c in range(NC):
        for b in range(NBLK):
            assign[c].extend(bucket_nodes[c * NBLK + b])
    assign = [np.array(a, np.int64) for a in assign]
    # chunked AG row layout: node at (core c, pos p) lands at
    # (p//CROWS)*(NC*CROWS) + c*CROWS + p%CROWS in xl_full [N, H]
    rowof = np.empty(N, np.int64)
    posof = np.empty(N, np.int64)
    coreof = np.empty(N, np.int64)
    for c in range(NC):
        p = np.arange(NLOC)
        rowof[assign[c]] = (p // CROWS) * (NC * CROWS) + c * CROWS + (p % CROWS)
        posof[assign[c]] = p
        coreof[assign[c]] = c
    percore = []
    for c in range(NC):
        sel = coreof[dst] == c
        s_, d_ = rowof[src[sel]], posof[dst[sel]]
        # within each dst block, order edges by required AG chunk of the src
        # row so leading tiles depend only on early chunks (split gathers)
        o = np.lexsort((s_ // (NC * CROWS), d_ // BLK))
        s_, d_ = s_[o], d_[o]
        blocks = []
        for bb in range(NBLK):
            m = (d_ // BLK) == bb
            blocks.append((s_[m], d_[m] - bb * BLK))
        percore.append(blocks)
    TBs = tuple(max(max(-(-len(percore[c][b][0]) // 128), 1) for c in range(NC))
                for b in range(NBLK))
    cum = np.concatenate([[0], np.cumsum(TBs)]).astype(int)
    NT = int(cum[-1])
    EPAD = NT * 128
    # per block: shared (min-over-cores) tile counts whose srcs all sit in
    # AG chunks <= ch; GB[b] is a nondecreasing list of 5 tile boundaries
    GB = []
    for b in range(NBLK):
        gb = []
        for ch in range(CH):
            lim = (ch + 1) * NC * CROWS
            cnt = min(int((percore[c][b][0] < lim).sum()) // 128
                      for c in range(NC))
            gb.append(cnt)
        gb[-1] = TBs[b]
        GB.append(tuple(gb))
    cores = []
    for c in range(NC):
        src16 = np.zeros(EPAD, np.int16)
        e01 = np.zeros((128, NT * BLK), npf8)
        e01T = np.zeros((128, NT * 128), npf8)
        for b in range(NBLK):
            s, d = percore[c][b]
            n = len(s)
            base = int(cum[b]) * 128
            src16[base:base + n] = s
            tt = int(cum[b]) + np.arange(n) // 128
            pp = np.arange(n) % 128
            e01[pp, tt * BLK + d] = 1.0
            e01T[d, tt * 128 + pp] = 1.0
        w = src16.reshape(-1, 16).T.copy()
        cores.append(dict(src16=np.tile(w, (8, 1)).copy(),
                          e01=np.ascontiguousarray(e01),
                          e01T=np.ascontiguousarray(e01T)))
    return (TBs, tuple(GB)), cores, assign


def _pack_weights(inp, perms):
    """Host-side full weight prep: beta-scaled bf16 blob + small f32 cols."""
    def pk(W):
        nk = W.shape[0] // 128
        return W.reshape(nk, 128, W.shape[1]).transpose(1, 0, 2).reshape(
            nk * 128, W.shape[1])

    wbf = np.zeros((WBF_ROWS, H), npbf)
    wf = np.zeros((128, WF_COLS), np.float32)
    for l in range(4):
        rowp = perms[l - 1] if l > 0 else None
        att = np.asarray(inp[f"att{l + 1}"], np.float32)[perms[l]]
        beta = np.maximum(np.abs(att), 1e-30)
        bb = np.asarray(inp[f"b{l + 1}"], np.float32)[perms[l]]
        bl = np.asarray(inp[f"bl{l + 1}"], np.float32)[perms[l]]
        br = np.asarray(inp[f"br{l + 1}"], np.float32)[perms[l]]
        for nm, bias in (("wl", bl + bb), ("wr", br - bb)):
            W = np.asarray(inp[f"{nm}{l + 1}"], np.float32)
            if rowp is not None:
                W = W[rowp, :]
            W = W[:, perms[l]] * beta[None, :]
            wbf[OFF[f"{nm}{l}"]:OFF[f"{nm}{l}"] + W.shape[0], :] = pk(W)
            off = OFF["blb" if nm == "wl" else "brb"] + l
            wbf[off, :] = bias * beta
        wf[:, WF_RECIP + 4 * l:WF_RECIP + 4 * l + 4] = (
            1.0 / beta).reshape(4, 128).T
    wbf[OFF["lw1"]:OFF["lw1"] + H, :] = pk(
        np.asarray(inp["lw1"], np.float32)[perms[3], :])
    wbf[OFF["lw2"]:OFF["lw2"] + H, :256] = pk(np.asarray(inp["lw2"], np.float32))
    wbf[OFF["lw3"]:OFF["lw3"] + 256, :2] = pk(np.asarray(inp["lw3"], np.float32))
    wf[:, WF_LB1:WF_LB1 + 4] = np.asarray(inp["lb1"], np.float32).reshape(4, 128).T
    wf[:, WF_LB2:WF_LB2 + 2] = np.asarray(inp["lb2"], np.float32).reshape(2, 128).T
    wf[0:2, WF_LB3] = np.asarray(inp["lb3"], np.float32)
    return wbf, wf


# -------------------------------------------------------------- bass program
def _build(TB_info, KP, single_core=False, nlayers=4, nedge=True, reps=1,
           l0ag=False):
    TBs, GB = TB_info
    TBs = tuple(TBs)
    TBMAX = max(TBs)
    cum = [0]
    for t in TBs:
        cum.append(cum[-1] + t)
    NT = cum[-1]
    nc = bacc.Bacc("TRN2", num_swdge_queues=2)
    P = nc.declare_dram_parameter
    xf_in = P("xf", [DIN, N], bf16, isOutput=False)
    x_in = P("x", [DIN, NLOC], bf16, isOutput=False)
    wb_in = P("wbf", [WBF_ROWS, H], bf16, isOutput=False)
    wf_in = P("wf32", [128, WF_COLS], f32, isOutput=False)
    srcidx_in = P("srcidx", [128, NT * 8], i16, isOutput=False)
    e01_in = P("e01", [128, NT * BLK], f8, isOutput=False)
    e01T_in = P("e01T", [128, NT * 128], f8, isOutput=False)
    ident_in = P("ident", [128, 128], bf16, isOutput=False)
    logitsT_out = P("logitsT", [2, NLOC], f32, isOutput=True)
    probs0_out = P("probs0", [1, NLOC], f32, isOutput=True)
    probs1_out = P("probs1", [1, NLOC], f32, isOutput=True)

    xl_loc = [nc.dram_tensor(f"xlloc{l}", [NLOC, H], bf16) for l in range(4)]
    xl_full = [nc.dram_tensor(f"xlfull{l}", [N, H], bf16, addr_space="Shared")
               for l in range(4)]

    with tile.TileContext(nc) as tc, ExitStack() as ctx:
        wp = ctx.enter_context(tc.tile_pool(name="wp", bufs=1))
        np_ = ctx.enter_context(tc.tile_pool(name="np", bufs=2))
        np2 = ctx.enter_context(tc.tile_pool(name="np2", bufs=2))
        ep = ctx.enter_context(tc.tile_pool(name="ep", bufs=2))
        gp = ctx.enter_context(tc.tile_pool(name="gp", bufs=2))
        ps = ctx.enter_context(tc.tile_pool(name="ps", bufs=1, space="PSUM"))
        ps2 = ctx.enter_context(tc.tile_pool(name="ps2", bufs=2, space="PSUM"))

        # ---------------- constants ----------------
        ones128 = wp.tile([128, 1], bf16, tag="ones128")
        nc.vector.memset(ones128[:, :], 1.0)
        onesrow = wp.tile([1, 128], bf16, tag="onesrow")
        nc.vector.memset(onesrow[:1, :], 1.0)
        sgn = wp.tile([2, 1], f32, tag="sgn")
        nc.vector.memset(sgn[:2, :], 1.0)
        nc.vector.tensor_scalar_mul(sgn[0:1, :], sgn[0:1, :], -1.0)
        ident = wp.tile([128, 128], bf16, tag="ident")
        nc.scalar.dma_start(out=ident[:, :], in_=ident_in[:, :])

        # ---------------- weights (all host-prepped, straight DMA loads) -----
        # layer-0 weights on the sync queue (first need), rest on scalar
        wld_t, wrd_t, blb_row, brb_row, recipcol = [], [], [], [], []
        for l in range(4):
            din = DIN if l == 0 else H
            nk0 = din // 128
            weng = nc.sync if l == 0 else nc.scalar
            wld = wp.tile([128, nk0, H], bf16, tag=f"wld{l}")
            wrd = wp.tile([128, nk0, H], bf16, tag=f"wrd{l}")
            wld_t.append(wld)
            wrd_t.append(wrd)
            for W_off, wdev in ((OFF[f"wl{l}"], wld), (OFF[f"wr{l}"], wrd)):
                weng.dma_start(out=wdev[:, :, :],
                               in_=wb_in[W_off:W_off + nk0 * 128, :].rearrange(
                                   "(p k) h -> p k h", p=128))
            t = wp.tile([1, H], bf16, tag=f"blb{l}")
            weng.dma_start(out=t[:1, :], in_=wb_in[OFF["blb"] + l:OFF["blb"] + l + 1, :])
            blb_row.append(t)
            t = wp.tile([1, H], bf16, tag=f"brb{l}")
            weng.dma_start(out=t[:1, :], in_=wb_in[OFF["brb"] + l:OFF["brb"] + l + 1, :])
            brb_row.append(t)
        wfcols = wp.tile([128, WF_COLS], f32, tag="wfcols")
        nc.sync.dma_start(out=wfcols[:, :], in_=wf_in[:, :])
        recipcol = [wfcols[:, WF_RECIP + 4 * l:WF_RECIP + 4 * l + 4]
                    for l in range(4)]
        lb1col = wfcols[:, WF_LB1:WF_LB1 + 4]
        lb2col = wfcols[:, WF_LB2:WF_LB2 + 2]
        lb3col = wfcols[:, WF_LB3:WF_LB3 + 1]

        # ---------------- MLP weights ----------------
        lw1_dev = wp.tile([128, 4, H], bf16, tag="lw1")
        nc.scalar.dma_start(out=lw1_dev[:, :, :],
                            in_=wb_in[OFF["lw1"]:OFF["lw1"] + H, :].rearrange(
                                "(p k) h -> p k h", p=128))
        lw2_dev = wp.tile([128, 4, 256], bf16, tag="lw2")
        nc.scalar.dma_start(out=lw2_dev[:, :, :],
                            in_=wb_in[OFF["lw2"]:OFF["lw2"] + H, :256].rearrange(
                                "(p k) h -> p k h", p=128))
        lw3_dev = wp.tile([128, 2, 2], bf16, tag="lw3")
        nc.scalar.dma_start(out=lw3_dev[:, :, :],
                            in_=wb_in[OFF["lw3"]:OFF["lw3"] + 256, :2].rearrange(
                                "(p k) h -> p k h", p=128))

        # edge-phase constants: big loads deferred here so the sync queue
        # serves layer-0 weights/x first (edge_a needs these only after the
        # first node blocks).
        srcidx = wp.tile([128, NT * 8], i16, tag="srcidx")
        nc.scalar.dma_start(out=srcidx[:, :], in_=srcidx_in[:, :])
        e01T_sb = wp.tile([128, NT * 128], f8, tag="e01T")
        nc.scalar.dma_start(out=e01T_sb[:, :], in_=e01T_in[:, :])
        e01_sb = wp.tile([128, NT * BLK], f8, tag="e01")
        nc.scalar.dma_start(out=e01_sb[:, :], in_=e01_in[:, :])

        # xr for the current/next layer's own nodes (rotation across layers)
        xr_cur = [np2.tile([128, NBLK, H], bf16, tag="xr", name=f"xr{i}")
                  for i in range(2)]

        # ---------------- helpers ----------------
        def node_block(l, b, lhsT_fn, nk):
            """xl/xr for dst block b of layer l from feature-major lhsT chunks."""
            pxl = ps.tile([128, H], f32, tag="pnl", bufs=2)
            pxr = ps.tile([128, H], f32, tag="pnr")
            for k in range(nk):
                lhsT = lhsT_fn(k)
                nc.tensor.matmul(pxl[:BLK, :], lhsT, wld_t[l][:, k, :],
                                 start=(k == 0), stop=False, skip_group_check=True)
                nc.tensor.matmul(pxr[:BLK, :], lhsT, wrd_t[l][:, k, :],
                                 start=(k == 0), stop=False, skip_group_check=True)
            nc.tensor.matmul(pxl[:BLK, :], onesrow[:1, :BLK], blb_row[l][:1, :],
                             start=False, stop=True, skip_group_check=True)
            nc.tensor.matmul(pxr[:BLK, :], onesrow[:1, :BLK], brb_row[l][:1, :],
                             start=False, stop=True, skip_group_check=True)
            xl_blk = np_.tile([128, H], bf16, tag="xlblk", bufs=2)
            nc.vector.tensor_copy(xl_blk[:BLK, :], pxl[:BLK, :])
            nc.vector.tensor_copy(xr_cur[l % 2][:BLK, b, :], pxr[:BLK, :])
            eng = nc.sync if b % 2 == 0 else nc.scalar
            eng.dma_start(out=xl_loc[l][b * BLK:(b + 1) * BLK, :],
                          in_=xl_blk[:BLK, :])

        def ag_chunk(l, ch):
            if single_core:
                for cc in range(NC):
                    eng = nc.sync if cc % 2 == 0 else nc.scalar
                    eng.dma_start(
                        out=xl_full[l][ch * NC * CROWS + cc * CROWS:
                                       ch * NC * CROWS + (cc + 1) * CROWS, :],
                        in_=xl_loc[l][ch * CROWS:(ch + 1) * CROWS, :])
            else:
                nc.gpsimd.collective_compute(
                    "AllGather", ALU.bypass,
                    replica_groups=[list(range(NC))],
                    ins=[xl_loc[l][ch * CROWS:(ch + 1) * CROWS, :]],
                    outs=[xl_full[l][ch * NC * CROWS:(ch + 1) * NC * CROWS, :]],
                )

        def edge_a(l, b):
            """Stage A for dst block b: split gathers (leading tiles depend
            only on early AG chunks); per tile, accumulate u = xr[dst]
            (one-hot matmul) + xlg (ident matmul) in PSUM, one full-width
            Prelu with fused accum row-sum (S_all) + DVE reduce of the
            neg-att slab; e = S_all - 2*S_neg. Returns (xlg, esum)."""
            TB = TBs[b]
            c0 = cum[b]
            kp = KP[l]
            # block 0 gets a dedicated buffer so its gather needn't wait for
            # the previous layer's last blocks to release a pool slot
            if b == 0:
                xlg = gp.tile([128, TBMAX, H], bf16, tag="xlg0", bufs=2)
            else:
                xlg = gp.tile([128, TBMAX, H], bf16, tag="xlg", bufs=3)
            # split gathers by required AG chunk: fine (5-way) for layer 0
            # where the serial AG chain is exposed, coarse (early/late)
            # after; each dma_gather costs ~1us of Q7 descgen.
            if l == 0:
                splits = [(GB[b][ch], ch) for ch in range(CH)]
            else:
                splits = [(GB[b][CH - 2], CH - 2), (GB[b][CH - 1], CH - 1)]
            t0 = 0
            for t1, ch in splits:
                if t1 <= t0:
                    continue
                rows = (ch + 1) * NC * CROWS
                nc.gpsimd.dma_gather(
                    out_ap=xlg[:, t0:t1, :], in_ap=xl_full[l][0:rows, :],
                    idxs_ap=srcidx[:, (c0 + t0) * 8:(c0 + t1) * 8],
                    num_idxs=(t1 - t0) * 128, num_idxs_reg=(t1 - t0) * 128,
                    elem_size=H, single_packet=False, queue_num=b % 2)
                t0 = t1
            es_all = ep.tile([128, TBMAX], f32, tag="esall", bufs=2)
            es_neg = ep.tile([128, TBMAX], f32, tag="esneg", bufs=2)
            for t in range(TB):
                pbc = ps2.tile([128, H], f32, tag="pbc")
                nc.tensor.matmul(pbc[:, :],
                                 e01T_sb[:BLK, (c0 + t) * 128:(c0 + t + 1) * 128],
                                 xr_cur[l % 2][:BLK, b, :],
                                 start=True, stop=False, skip_group_check=True)
                nc.tensor.matmul(pbc[:, :], ident[:, :], xlg[:, t, :],
                                 start=False, stop=True, skip_group_check=True)
                wscr = ep.tile([128, H], bf16, tag="wscr", bufs=2)
                nc.scalar.activation(wscr[:, :], pbc[:, :], AF.Prelu, alpha=NEG,
                                     accum_out=es_all[:, t:t + 1])
                if kp < H:
                    nc.vector.tensor_reduce(es_neg[:, t:t + 1], wscr[:, kp:H],
                                            axis=mybir.AxisListType.X, op=ALU.add)
            # esum + exp in two halves so stage B's scatter can start on the
            # first half while the second half's accums still finish
            esum = ep.tile([128, TBMAX], f32, tag="eavg", bufs=2)
            pbuf = ep.tile([128, TBMAX], f32, tag="pbuf", bufs=2)
            th = (TB + 1) // 2
            for t0e, t1e in ((0, th), (th, TB)):
                if t1e <= t0e:
                    continue
                if kp == H:
                    nc.scalar.activation(pbuf[:, t0e:t1e], es_all[:, t0e:t1e],
                                         AF.Exp)
                else:
                    nc.vector.tensor_scalar_mul(es_neg[:, t0e:t1e],
                                                es_neg[:, t0e:t1e], -2.0)
                    nc.vector.tensor_add(esum[:, t0e:t1e], es_all[:, t0e:t1e],
                                         es_neg[:, t0e:t1e])
                    nc.scalar.activation(pbuf[:, t0e:t1e], esum[:, t0e:t1e],
                                         AF.Exp)
            return xlg, pbuf

        def edge_b(l, b, ctx, hT_out):
            """Stage B: exp, scatter-matmul aggregation, relu (DVE), transposes
            with 1/beta fold into hT_out."""
            TB = TBs[b]
            c0 = cum[b]
            xlg, pbuf = ctx
            pf = ps.tile([128, H], f32, tag="pf")
            ps1 = ps.tile([128, 1], f32, tag="ps1")
            for t in range(TB):
                S = ep.tile([128, BLK], bf16, tag="S")
                nc.vector.tensor_scalar_mul(
                    S[:, :], e01_sb[:, (c0 + t) * BLK:(c0 + t + 1) * BLK],
                    pbuf[:, t:t + 1])
                nc.tensor.matmul(pf[:BLK, :], S[:, :], xlg[:, t, :],
                                 start=(t == 0), stop=(t == TB - 1),
                                 skip_group_check=True)
                nc.tensor.matmul(ps1[:BLK, :1], S[:, :], ones128[:, :1],
                                 start=(t == 0), stop=(t == TB - 1),
                                 skip_group_check=True)
            srec = ep.tile([128, 1], f32, tag="srec")
            nc.vector.reciprocal(srec[:BLK, :], ps1[:BLK, :1])
            hb = ep.tile([128, H], bf16, tag="hb", bufs=1)
            nc.vector.tensor_scalar(hb[:BLK, :], pf[:BLK, :], srec[:BLK, :], 0.0,
                                    op0=ALU.mult, op1=ALU.max)
            for kc in range(4):
                ptr = ps2.tile([128, 128], bf16, tag="ptr", bufs=1)
                nc.tensor.transpose(ptr[:, :BLK], hb[:BLK, ts(kc, 128)],
                                    ident[:BLK, :BLK])
                nc.vector.tensor_scalar_mul(hT_out[:, kc, b * BLK:(b + 1) * BLK],
                                            ptr[:, :BLK], recipcol[l][:, kc:kc + 1])

        # ---------------- MLP head (per 128-node chunk) ----------------
        jchunks = [(j0, min(128, NLOC - j0)) for j0 in range(0, NLOC, 128)]
        mlp_ready_at = {}
        for _j, (_j0, _w) in enumerate(jchunks):
            mlp_ready_at.setdefault((_j0 + _w - 1) // BLK, []).append(_j)

        def mlp_chunk(jidx, hT):
            j0, w = jchunks[jidx]
            h1c = np_.tile([128, 4, 128], bf16, tag="h1c", bufs=2)
            for m in range(4):
                pm = ps2.tile([128, H], f32, tag="pbc")
                for k in range(4):
                    nc.tensor.matmul(pm[:, :w], lw1_dev[:, k, ts(m, 128)],
                                     hT[:, k, j0:j0 + w], start=(k == 0),
                                     stop=(k == 3), skip_group_check=True)
                nc.scalar.activation(h1c[:, m, :w], pm[:, :w], AF.Relu,
                                     bias=lb1col[:, m:m + 1])
            h2c = np_.tile([128, 2, 128], bf16, tag="h2c", bufs=2)
            for m in range(2):
                pm = ps2.tile([128, H], f32, tag="pbc")
                for k in range(4):
                    nc.tensor.matmul(pm[:, :w], lw2_dev[:, k, ts(m, 128)],
                                     h1c[:, k, :w], start=(k == 0),
                                     stop=(k == 3), skip_group_check=True)
                nc.scalar.activation(h2c[:, m, :w], pm[:, :w], AF.Relu,
                                     bias=lb2col[:, m:m + 1])
            pm3 = ps2.tile([128, H], f32, tag="pbc")
            for k in range(2):
                nc.tensor.matmul(pm3[:2, :w], lw3_dev[:, k, :],
                                 h2c[:, k, :w], start=(k == 0), stop=(k == 1),
                                 skip_group_check=True)
            logc = np_.tile([2, 128], f32, tag="logc", bufs=2)
            nc.scalar.activation(logc[:2, :w], pm3[:2, :w], AF.Identity,
                                 bias=lb3col[:2, :])
            pd = ps2.tile([128, H], f32, tag="pbc")
            nc.tensor.matmul(pd[:1, :w], sgn[:2, :], logc[:2, :w],
                             start=True, stop=True, skip_group_check=True)
            emd = np_.tile([1, 128], f32, tag="emd", bufs=2)
            nc.scalar.activation(emd[:1, :w], pd[:1, :w], AF.Exp, scale=-1.0)
            p1c = np_.tile([1, 128], f32, tag="p1c", bufs=2)
            nc.vector.tensor_scalar_add(p1c[:1, :w], emd[:1, :w], 1.0)
            nc.vector.reciprocal(p1c[:1, :w], p1c[:1, :w])
            nc.vector.tensor_mul(emd[:1, :w], p1c[:1, :w], emd[:1, :w])
            p0c = emd
            nc.scalar.dma_start(out=logitsT_out[:, j0:j0 + w], in_=logc[:2, :w])
            nc.scalar.dma_start(out=probs0_out[:, j0:j0 + w], in_=p0c[:1, :w])
            nc.scalar.dma_start(out=probs1_out[:, j0:j0 + w], in_=p1c[:1, :w])

        # ---------------- main ----------------
        for rep in range(reps):
            # layer 0: every core computes xl for ALL nodes from the
            # replicated input x (no layer-0 AllGather at all) straight into
            # its local xl_full[0]; xr only for its own dst nodes from the
            # small per-core x.
            if l0ag:
                # layer 0 node phase for own nodes only + AllGather chunks
                for b in range(NBLK):
                    xc = ep.tile([128, 8, BLK], bf16, tag="xc", bufs=2)
                    nc.sync.dma_start(
                        out=xc[:, :, :],
                        in_=x_in[:, b * BLK:(b + 1) * BLK].rearrange(
                            "(k p) n -> p k n", p=128))
                    node_block(0, b, lambda k, _xc=xc: _xc[:, k, :BLK], 8)
                    if (b + 1) % CBLK == 0:
                        ag_chunk(0, b // CBLK)
            else:
                # own xr blocks from the per-core x (needed first by stage A)
                for b in range(NBLK):
                    xc = ep.tile([128, 8, BLK], bf16, tag="xc", bufs=2)
                    nc.sync.dma_start(
                        out=xc[:, :, :],
                        in_=x_in[:, b * BLK:(b + 1) * BLK].rearrange(
                            "(k p) n -> p k n", p=128))
                    pxr = ps.tile([128, H], f32, tag="pnr")
                    for k in range(8):
                        nc.tensor.matmul(pxr[:BLK, :], xc[:, k, :BLK],
                                         wrd_t[0][:, k, :], start=(k == 0),
                                         stop=False, skip_group_check=True)
                    nc.tensor.matmul(pxr[:BLK, :], onesrow[:1, :BLK],
                                     brb_row[0][:1, :], start=False, stop=True,
                                     skip_group_check=True)
                    nc.vector.tensor_copy(xr_cur[0][:BLK, b, :], pxr[:BLK, :])
                # xl for ALL nodes from the replicated xf (no layer-0 AG)
                NGB = N // BLK  # 80 global blocks
                for gb in range(NGB):
                    xc = ep.tile([128, 8, BLK], bf16, tag="xc", bufs=2)
                    nc.sync.dma_start(
                        out=xc[:, :, :],
                        in_=xf_in[:, gb * BLK:(gb + 1) * BLK].rearrange(
                            "(k p) n -> p k n", p=128))
                    pxl = ps.tile([128, H], f32, tag="pnl", bufs=2)
                    for k in range(8):
                        nc.tensor.matmul(pxl[:BLK, :], xc[:, k, :BLK],
                                         wld_t[0][:, k, :], start=(k == 0),
                                         stop=False, skip_group_check=True)
                    nc.tensor.matmul(pxl[:BLK, :], onesrow[:1, :BLK],
                                     blb_row[0][:1, :], start=False, stop=True,
                                     skip_group_check=True)
                    xl_blk = np_.tile([128, H], bf16, tag="xlblk", bufs=2)
                    if gb % 2 == 0:
                        nc.vector.tensor_copy(xl_blk[:BLK, :], pxl[:BLK, :])
                    else:
                        nc.scalar.activation(xl_blk[:BLK, :], pxl[:BLK, :], AF.Copy)
                    eng = nc.sync if gb % 2 == 0 else nc.scalar
                    eng.dma_start(out=xl_full[0][gb * BLK:(gb + 1) * BLK, :],
                                  in_=xl_blk[:BLK, :])
            hT = None
            for l in range(nlayers):
                if not nedge:
                    continue
                hT_next = np2.tile([128, 4, NLOC], bf16, tag="hT", name=f"hT{rep}_{l}")

                def finish(b, _l=l, _hT=hT_next):
                    edge_b(_l, b, ctxs[b], _hT)
                    if _l + 1 < nlayers:
                        node_block(_l + 1, b,
                                   lambda k, _h=_hT, _b=b:
                                   _h[:, k, _b * BLK:(_b + 1) * BLK], 4)
                        if (b + 1) % CBLK == 0:
                            ag_chunk(_l + 1, b // CBLK)
                    else:
                        for j in mlp_ready_at.get(b, []):
                            mlp_chunk(j, _hT)

                ctxs = {}
                for b in range(NBLK):
                    ctxs[b] = edge_a(l, b)
                    if b > 0:
                        finish(b - 1)
                finish(NBLK - 1)
                hT = hT_next

            # ---------------- MLP head (fallback when no edge layers ran) ----
            if hT is None:
                hT = np2.tile([128, 4, NLOC], bf16, tag="hT", name=f"hT{rep}_x")
                nc.vector.memset(hT[:, :, :], 0.0)
                for j in range(len(jchunks)):
                    mlp_chunk(j, hT)

    nc.compile()
    return nc


_CACHE = {}
_LAST_IN_MAPS = None


def _get_program(TBs, KP):
    key = (tuple(TBs), tuple(KP))
    if key not in _CACHE:
        _CACHE[key] = _build(TBs, KP)
    return _CACHE[key]


def _run(inputs, trace=False):
    inp = {k: np.asarray(v) for k, v in inputs.items()}
    x = inp["x"].astype(np.float32)
    edge_index = inp["edge_index"].astype(np.int64)
    TBs, cores, assign = _prep_edges(edge_index)

    perms, KP = [], []
    for l in range(1, 5):
        att = inp[f"att{l}"].astype(np.float32)
        perm = np.argsort(att <= 0, kind="stable")
        perms.append(perm)
        KP.append(int((att > 0).sum()))
    wbf, wf = _pack_weights(inp, perms)
    ident = np.eye(128, dtype=npbf)

    # xf: x columns in global xl_full row order, replicated to every core
    rowof = np.empty(N, np.int64)
    for c in range(NC):
        p = np.arange(NLOC)
        rowof[assign[c]] = (p // CROWS) * (NC * CROWS) + c * CROWS + (p % CROWS)
    node_at_row = np.empty(N, np.int64)
    node_at_row[rowof] = np.arange(N)
    xf = np.ascontiguousarray(x[node_at_row].T.astype(npbf))

    ncprog = _get_program(TBs, KP)
    in_maps = []
    for c in range(NC):
        xT = np.ascontiguousarray(x[assign[c]].T.astype(npbf))
        m = {"x": xT, "xf": xf, "wbf": wbf, "wf32": wf,
             "srcidx": cores[c]["src16"],
             "e01": cores[c]["e01"], "e01T": cores[c]["e01T"], "ident": ident}
        in_maps.append(m)

    global _LAST_IN_MAPS
    _LAST_IN_MAPS = in_maps
    res = run_bass_kernel_spmd(ncprog, in_maps, list(range(NC)), trace=trace)
    logits = np.empty((N, 2), np.float32)
    probs = np.empty((N, 2), np.float32)
    for c in range(NC):
        r = res.results[c]
        logits[assign[c]] = r["logitsT"].T
        probs[assign[c], 0] = r["probs0"][0]
        probs[assign[c], 1] = r["probs1"][0]
    return (logits, probs), res


def kernel(**inputs):
    out, _ = _run(inputs, trace=False)
    return out

